# revision 1
# baseline (speedup 1.0000x reference)
"""Gated linear attention kernel for one TRN2 chip (8 NeuronCores).

Math (see reference):
    q = elu(X Wq)+1, k = elu(X Wk)+1, v = X Wv, g = X Wg
    qk = sum_d(q*k) per head; k_sum = sum_seq(k); norm = sum_d(q*k_sum)
    z = qk*v/(norm+1e-6); z = LayerNorm(z)*gamma+beta; out = (z*silu(g)) Wo

Sharding: data-parallel over the 16384 tokens, 2048 per core; cores 0-3 own
batch 0, cores 4-7 batch 1.  The only cross-core coupling is k_sum (a [1,1024]
vector per batch) -> AllReduce within 4-core groups, overlapped with the
v/g projections.

Layout: activations token-major [tok, feat].  X is transposed on the host so
projection matmuls get their stationary operand (X^T k-slices) directly.
All matmuls run in float32r (full PE rate at moving-dim >= 256).
elu(x)+1 == min(exp(x), 1) + relu(x) exactly, giving a 2-ACT + 1-DVE
implementation.  gamma is folded into Wo on the host; beta==0 is verified on
the host (a broadcast-add is traced in only when beta is nonzero).
"""

import os

import numpy as np

import concourse.bass as bass
import concourse.mybir as mybir
import concourse.tile as tile
from concourse.bass_utils import run_bass_kernel_spmd
from concourse.masks import make_identity

F32 = mybir.dt.float32
F32R = mybir.dt.float32r
AX = mybir.AxisListType
ALU = mybir.AluOpType
ACT_F = mybir.ActivationFunctionType

H = 1024
NH = 16
DK = 64
N_CORES = 8


def _split_multi_waits(nc, cap=1):
    """walrus in this image rejects instructions with more than ~2 sync waits
    (Tile attaches several to its kernel-tail drain).  Move excess waits onto
    preceding same-engine NoOps."""
    for f in nc.m.functions:
        for bb in f.blocks:
            insts = bb.instructions
            new_list = []
            changed = False
            for inst in insts:
                si = inst.sync_info
                waits = list(si.on_wait) if si else []
                if len(waits) > cap:
                    changed = True
                    for kk, w in enumerate(waits[:-cap]):
                        new_list.append(
                            mybir.InstNoOp(
                                name=f"{inst.name}-wsplit{kk}",
                                engine=inst.engine,
                                ins=[],
                                outs=[],
                                sync_info=mybir.SyncInfo(on_wait=[w], on_update=[]),
                            )
                        )
                    inst.sync_info = mybir.SyncInfo(
                        on_wait=waits[-cap:], on_update=list(si.on_update)
                    )
                new_list.append(inst)
            if changed:
                live = bb.instructions
                live.clear()
                for i in new_list:
                    bb.add_instruction(i)
    return nc


def build_gla(T=2048, groups=((0, 1, 2, 3), (4, 5, 6, 7)), n_devices=8,
              apply_beta=False, split_waits=True, use_silu=True):
    """Build the per-core SPMD program.  T = tokens per core."""
    assert T % 512 == 0
    NT = T // 128      # 128-token tiles
    KT = H // 128      # contraction slices

    nc = bass.Bass(num_devices=n_devices)
    xt_d = nc.declare_dram_parameter("xt", [H, T], F32, isOutput=False)
    w_d = {
        n: nc.declare_dram_parameter(n, [H, H], F32, isOutput=False)
        for n in ("wq", "wk", "wv", "wg", "wo")
    }
    beta_d = (
        nc.declare_dram_parameter("beta", [1, H], F32, isOutput=False)
        if apply_beta
        else None
    )
    out_d = nc.declare_dram_parameter("out", [T, H], F32, isOutput=True)

    qsp = nc.dram_tensor("q_spill", [T, H], F32)
    ksp = nc.dram_tensor("k_spill", [T, H], F32)
    ks_in = nc.dram_tensor("ks_in", [1, H], F32)
    ks_out = nc.dram_tensor("ks_out", [1, H], F32)

    def mm(ps, lhsT, rhs, start, stop):
        nc.tensor.matmul(ps, lhsT=lhsT, rhs=rhs, start=start, stop=stop)

    with tile.TileContext(nc) as tc:
        with (
            tc.tile_pool(name="singles", bufs=1) as singles,
            tc.tile_pool(name="w", bufs=3) as wpool,
            tc.tile_pool(name="xtb", bufs=2) as xpool,
            tc.tile_pool(name="qk", bufs=2) as qkpool,      # per-tile q/k/s/z/u
            tc.tile_pool(name="small", bufs=3) as smpool,
            tc.tile_pool(name="ut", bufs=2) as utpool,
            tc.tile_pool(name="pa", bufs=2, space="PSUM") as pa,
            tc.tile_pool(name="pb", bufs=2, space="PSUM") as pb,
        ):
            ident = singles.tile([128, 128], F32)
            make_identity(nc, ident)
            ones_f32 = singles.tile([128, 1], F32)
            nc.vector.memset(ones_f32, 1.0)
            ones_col = singles.tile([128, 1], F32R)
            nc.vector.tensor_copy(out=ones_col, in_=ones_f32)
            eps_ln = singles.tile([128, 1], F32)
            nc.vector.memset(eps_ln, 1e-5)
            qk_all = singles.tile([128, NT, NH], F32)

            def alloc_w():
                return wpool.tile([128, KT, H], F32R, tag="w", name="wslot")

            def load_w_slice(t, name, k):
                nc.sync.dma_start(out=t[:, k, :],
                                  in_=w_d[name][128 * k:128 * (k + 1), :].bitcast(F32R))

            def load_xt_block(b):
                # [128, KT, 512]: k-slices of X^T for tokens 512b..512b+512
                t = xpool.tile([128, KT, 512], F32R, tag="xtb")
                for k in range(KT):
                    nc.sync.dma_start(
                        out=t[:, k, :],
                        in_=xt_d[128 * k:128 * (k + 1),
                                 512 * b:512 * (b + 1)].bitcast(F32R),
                    )
                return t

            def elu1(dst, ps, tmp_pool):
                # dst = elu(ps)+1 = min(exp(ps), 1) + relu(ps); ps is PSUM
                e = tmp_pool.tile([128, 512], F32, tag="elue")
                r = tmp_pool.tile([128, 512], F32, tag="elur")
                nc.scalar.activation(out=e, in_=ps, func=ACT_F.Exp)
                nc.scalar.activation(out=r, in_=ps, func=ACT_F.Relu)
                nc.vector.scalar_tensor_tensor(
                    out=dst, in0=e, scalar=1.0, in1=r,
                    op0=ALU.min, op1=ALU.add,
                )

            # -------- phase 1a: k projection + k_sum (k spilled to DRAM) ----
            # interleave the first xt block with wk so the first matmul's
            # k=0 inputs land after ~1 MB of DMA instead of 10 MB
            xtb0 = xpool.tile([128, KT, 512], F32R, tag="xtb")
            wk_t = alloc_w()
            for k in range(KT):
                nc.sync.dma_start(
                    out=xtb0[:, k, :],
                    in_=xt_d[128 * k:128 * (k + 1), 0:512].bitcast(F32R))
                load_w_slice(wk_t, "wk", k)
            wq_t = alloc_w()           # prefetched during phase 1a
            for k in range(KT):
                load_w_slice(wq_t, "wq", k)
            with tc.tile_pool(name="ks", bufs=1, space="PSUM") as kspool:
                ks_ps = kspool.tile([1, H], F32)
                xtb = None
                for t in range(NT):
                    b, sub = divmod(t, 4)
                    if sub == 0:
                        xtb = xtb0 if b == 0 else load_xt_block(b)
                    kt = qkpool.tile([128, H], F32R, tag="kt")
                    for n in range(2):
                        pk = pb.tile([128, 512], F32, tag="pb")
                        for k in range(KT):
                            lhs = xtb[:, k, 128 * sub:128 * (sub + 1)]
                            nsl = slice(512 * n, 512 * (n + 1))
                            mm(pk, lhs, wk_t[:, k, nsl], k == 0, k == KT - 1)
                        elu1(kt[:, 512 * n:512 * (n + 1)], pk, smpool)
                    # k_sum partial: ones^T @ k  (contract over tokens)
                    for n in range(2):
                        nc.tensor.matmul(
                            ks_ps[:, 512 * n:512 * (n + 1)],
                            lhsT=ones_col,
                            rhs=kt[:, 512 * n:512 * (n + 1)],
                            start=(t == 0), stop=(t == NT - 1),
                        )
                    nc.sync.dma_start(out=ksp[128 * t:128 * (t + 1), :],
                                      in_=kt.bitcast(F32))
                ks_sb = singles.tile([1, H], F32)
                nc.vector.tensor_copy(out=ks_sb, in_=ks_ps)
            # fire the AllReduce now -- phase 1b + the v/g matmuls of phase 2
            # cover its latency
            nc.sync.dma_start(out=ks_in[:, :], in_=ks_sb)
            nc.gpsimd.collective_compute(
                "AllReduce", ALU.add,
                replica_groups=[list(g) for g in groups],
                ins=[ks_in[:, :]], outs=[ks_out[:, :]],
            )
            ksb = singles.tile([128, H], F32)
            nc.gpsimd.dma_start(out=ksb, in_=ks_out[0:1, :].to_broadcast([128, H]))
            if apply_beta:
                beta_b = singles.tile([128, H], F32)
                nc.gpsimd.dma_start(out=beta_b,
                                    in_=beta_d[0:1, :].to_broadcast([128, H]))

            # -------- phase 1b: q projection + qk (q spilled to DRAM) -------
            xtb1_0 = xpool.tile([128, KT, 512], F32R, tag="xtb")
            wv_t = alloc_w()           # prefetched for phase 2
            for k in range(KT):
                nc.sync.dma_start(
                    out=xtb1_0[:, k, :],
                    in_=xt_d[128 * k:128 * (k + 1), 0:512].bitcast(F32R))
                load_w_slice(wv_t, "wv", k)
            xtb = None
            for t in range(NT):
                b, sub = divmod(t, 4)
                if sub == 0:
                    xtb = xtb1_0 if b == 0 else load_xt_block(b)
                qt = qkpool.tile([128, H], F32, tag="qt")
                for n in range(2):
                    pq = pa.tile([128, 512], F32, tag="pa")
                    for k in range(KT):
                        lhs = xtb[:, k, 128 * sub:128 * (sub + 1)]
                        nsl = slice(512 * n, 512 * (n + 1))
                        mm(pq, lhs, wq_t[:, k, nsl], k == 0, k == KT - 1)
                    elu1(qt[:, 512 * n:512 * (n + 1)], pq, smpool)
                ktr = qkpool.tile([128, H], F32, tag="kt")
                nc.sync.dma_start(out=ktr, in_=ksp[128 * t:128 * (t + 1), :])
                prod = qkpool.tile([128, H], F32, tag="prod")
                nc.vector.tensor_mul(prod, qt, ktr)
                nc.vector.reduce_sum(
                    out=qk_all[:, t, :],
                    in_=prod.rearrange("p (h d) -> p h d", d=DK),
                    axis=AX.X,
                )
                nc.sync.dma_start(out=qsp[128 * t:128 * (t + 1), :], in_=qt)

            # ---------------- phase 2: v, g, z, LN, gate, Wo ----------------
            wg_t = alloc_w()
            wo_t = alloc_w()
            xtb2_0 = xpool.tile([128, KT, 512], F32R, tag="xtb")
            for k in range(KT):
                nc.sync.dma_start(
                    out=xtb2_0[:, k, :],
                    in_=xt_d[128 * k:128 * (k + 1), 0:512].bitcast(F32R))
                load_w_slice(wg_t, "wg", k)
                load_w_slice(wo_t, "wo", k)
            with (
                tc.tile_pool(name="pt", bufs=2, space="PSUM") as ptpool,
                tc.tile_pool(name="py", bufs=2, space="PSUM") as pypool,
            ):
                xtb = None
                for t in range(NT):
                    b, sub = divmod(t, 4)
                    if sub == 0:
                        xtb = xtb2_0 if b == 0 else load_xt_block(b)
                    pvs = []
                    s_t = qkpool.tile([128, H], F32, tag="kt")
                    for n in range(2):
                        pv = pa.tile([128, 512], F32, tag="pa")
                        pg = pb.tile([128, 512], F32, tag="pb")
                        for k in range(KT):
                            lhs = xtb[:, k, 128 * sub:128 * (sub + 1)]
                            nsl = slice(512 * n, 512 * (n + 1))
                            mm(pv, lhs, wv_t[:, k, nsl], k == 0, k == KT - 1)
                            mm(pg, lhs, wg_t[:, k, nsl], k == 0, k == KT - 1)
                        ssl = s_t[:, 512 * n:512 * (n + 1)]
                        if use_silu:
                            nc.scalar.activation(out=ssl, in_=pg, func=ACT_F.Silu)
                        else:  # CoreSim has no Silu table
                            nc.scalar.activation(out=ssl, in_=pg,
                                                 func=ACT_F.Sigmoid)
                            nc.vector.tensor_mul(ssl, ssl, pg)
                        pvs.append(pv)
                    # normalizer = per-head dot(q, k_sum)
                    qt2 = qkpool.tile([128, H], F32, tag="qt")
                    nc.sync.dma_start(out=qt2, in_=qsp[128 * t:128 * (t + 1), :])
                    nprod = qkpool.tile([128, H], F32, tag="prod")
                    nc.vector.tensor_mul(nprod, qt2, ksb)
                    norm = smpool.tile([128, NH], F32, tag="norm")
                    nc.vector.reduce_sum(
                        out=norm, in_=nprod.rearrange("p (h d) -> p h d", d=DK),
                        axis=AX.X,
                    )
                    rec = smpool.tile([128, NH], F32, tag="rec")
                    nc.vector.tensor_scalar_add(out=rec, in0=norm, scalar1=1e-6)
                    nc.vector.reciprocal(out=rec, in_=rec)
                    r = smpool.tile([128, NH], F32, tag="r")
                    nc.vector.tensor_mul(r, qk_all[:, t, :], rec)
                    # z = r (broadcast over d) * v
                    z = qkpool.tile([128, H], F32, tag="z")
                    for n in range(2):
                        rs = r[:, 8 * n:8 * (n + 1)]
                        r_b = bass.AP(tensor=rs.tensor, offset=rs.offset,
                                      ap=[list(rs.ap[0]), list(rs.ap[1]), [0, DK]])
                        nc.vector.tensor_tensor(
                            out=z[:, 512 * n:512 * (n + 1)],
                            in0=pvs[n], in1=r_b, op=ALU.mult,
                        )
                    # LayerNorm over the full 1024 features
                    st = smpool.tile([128, 2, nc.vector.BN_STATS_DIM], F32, tag="bnst")
                    for n in range(2):
                        nc.vector.bn_stats(out=st[:, n, :],
                                           in_=z[:, 512 * n:512 * (n + 1)])
                    mv = smpool.tile([128, nc.vector.BN_AGGR_DIM], F32, tag="mv")
                    nc.vector.bn_aggr(out=mv, in_=st)
                    sd = smpool.tile([128, 1], F32, tag="sd")
                    nc.scalar.activation(out=sd, in_=mv[:, 1:2], func=ACT_F.Sqrt,
                                         bias=eps_ln)
                    nc.vector.reciprocal(out=sd, in_=sd)
                    nc.vector.tensor_scalar(
                        out=z, in0=z, scalar1=mv[:, 0:1], scalar2=sd,
                        op0=ALU.subtract, op1=ALU.mult,
                    )
                    if apply_beta:
                        nc.vector.tensor_add(out=z, in0=z, in1=beta_b)
                    u = z
                    nc.vector.tensor_mul(u, z, s_t)
                    # transpose u and apply Wo
                    ut = utpool.tile([128, KT, 128], F32R, tag="ut")
                    for k in range(KT):
                        pt = ptpool.tile([128, 128], F32, tag="pt")
                        nc.tensor.transpose(pt, u[:, 128 * k:128 * (k + 1)], ident)
                        nc.vector.tensor_copy(out=ut[:, k, :], in_=pt)
                    y_sb = qkpool.tile([128, H], F32, tag="y")
                    for n in range(2):
                        py = pypool.tile([128, 512], F32, tag="py")
                        for k in range(KT):
                            mm(py, ut[:, k, :],
                               wo_t[:, k, 512 * n:512 * (n + 1)],
                               k == 0, k == KT - 1)
                        nc.vector.tensor_copy(out=y_sb[:, 512 * n:512 * (n + 1)],
                                              in_=py)
                    nc.sync.dma_start(out=out_d[128 * t:128 * (t + 1), :],
                                      in_=y_sb)
    return _split_multi_waits(nc) if split_waits else nc


# ------------------------------------------------------------------
# host glue
# ------------------------------------------------------------------
_CACHE = {}
LAST_RESULT = None


def kernel(hidden_states, Wq, Wk, Wv, Wg, Wo, gamma, beta):
    hs = np.asarray(hidden_states, dtype=np.float32)
    Wq = np.asarray(Wq, dtype=np.float32)
    Wk = np.asarray(Wk, dtype=np.float32)
    Wv = np.asarray(Wv, dtype=np.float32)
    Wg = np.asarray(Wg, dtype=np.float32)
    Wo = np.asarray(Wo, dtype=np.float32)
    gamma = np.asarray(gamma, dtype=np.float32)
    beta = np.asarray(beta, dtype=np.float32)

    b, s, h = hs.shape
    tokens = hs.reshape(b * s, h)
    n_tok = b * s
    T = n_tok // N_CORES
    assert s % T == 0, "core token shards must not straddle batches"
    cores_per_batch = s // T

    groups = tuple(
        tuple(range(bi * cores_per_batch, (bi + 1) * cores_per_batch))
        for bi in range(b)
    )
    apply_beta = bool(np.any(beta))

    key = (T, groups, apply_beta)
    if key not in _CACHE:
        _CACHE[key] = build_gla(T=T, groups=groups, apply_beta=apply_beta)
    nc = _CACHE[key]

    wo_eff = np.ascontiguousarray(gamma[:, None] * Wo)
    in_maps = []
    for i in range(N_CORES):
        m = {
            "xt": np.ascontiguousarray(tokens[i * T:(i + 1) * T].T),
            "wq": Wq, "wk": Wk, "wv": Wv, "wg": Wg, "wo": wo_eff,
        }
        if apply_beta:
            m["beta"] = beta.reshape(1, h)
        in_maps.append(m)

    res = run_bass_kernel_spmd(
        nc, in_maps, core_ids=list(range(N_CORES)),
        trace=bool(os.environ.get("GLA_TRACE")),
    )
    global LAST_RESULT
    LAST_RESULT = res
    out = np.concatenate([res.results[i]["out"] for i in range(N_CORES)], axis=0)
    return out.reshape(b, s, h)



# revision 5
# speedup vs baseline: 1.2091x; 1.2091x over previous
"""Gated linear attention kernel for one TRN2 chip (8 NeuronCores).

Math (see reference):
    q = elu(X Wq)+1, k = elu(X Wk)+1, v = X Wv, g = X Wg
    qk = sum_d(q*k) per head; k_sum = sum_seq(k); norm = sum_d(q*k_sum)
    z = qk*v/(norm+1e-6); z = LayerNorm(z)*gamma+beta; out = (z*silu(g)) Wo

Sharding: data-parallel over the 16384 tokens, 2048 per core; cores 0-3 own
batch 0, cores 4-7 batch 1.  The only cross-core coupling is k_sum (a [1,1024]
vector per batch) -> AllReduce within 4-core groups, overlapped with the
q-projection phase.

Key layout decisions (v2, tuned against the perfetto trace of v1):
  * Everything the PE touches is bf16: X^T, the five weights, k, q, u.
    bf16 streams at the same 1 col/cycle as float32r but LDWEIGHTS is 2x
    faster (FWL) and DMA/SBUF cost halves.  PSUM accumulation stays fp32.
  * X^T (4 MB), k (4 MB) and q (4 MB) are SBUF-resident for the whole
    kernel -- no DRAM spills, X is DMA'd exactly once.  Total HBM traffic
    is 22 MB/core (was 84 MB), so the PE never waits on DMA.
  * elu(x)+1 == min(exp(x),1) + relu(x) exactly (2 ACT + 1 DVE op); exp and
    relu live in the same ACT table so phase 1 has no table reloads.
  * The LayerNorm rsqrt is computed on the DVE (Newton iteration seeded by
    the exponent bit-hack) and -- because 1/sigma is a per-token scalar that
    commutes with the Wo matmul -- applied to the Wo *output* during PSUM
    evacuation.  Phase 2's scalar engine runs only Silu+Copy (one table),
    eliminating the per-tile Silu<->Sqrt table thrash (42 us in v1).
  * Phase 2 is software-pipelined one tile deep: the transposes + Wo
    matmuls of tile t-1 are enqueued between the v/g matmuls of tile t so
    the PE never drains while the DVE normalization chain runs.
  * The 8 u-transposes of a tile write disjoint 128-col slices of ONE bf16
    PSUM bank, evacuated by a single scalar-engine copy.
gamma is folded into Wo on the host; beta==0 is verified on the host (the
slower beta path is only built when beta is nonzero).
"""

import os

import numpy as np

import concourse.bass as bass
import concourse.mybir as mybir
import concourse.tile as tile
from concourse.bass_utils import run_bass_kernel_spmd
from concourse.masks import make_identity

F32 = mybir.dt.float32
BF16 = mybir.dt.bfloat16
U32 = mybir.dt.uint32
AX = mybir.AxisListType
ALU = mybir.AluOpType
ACT_F = mybir.ActivationFunctionType

H = 1024
NH = 16
DK = 64
N_CORES = 8


def _split_multi_waits(nc, cap=1):
    """walrus in this image rejects instructions with more than ~2 sync waits
    (Tile attaches several to its kernel-tail drain).  Move excess waits onto
    preceding same-engine NoOps."""
    for f in nc.m.functions:
        for bb in f.blocks:
            insts = bb.instructions
            new_list = []
            changed = False
            for inst in insts:
                si = inst.sync_info
                waits = list(si.on_wait) if si else []
                if len(waits) > cap:
                    changed = True
                    for kk, w in enumerate(waits[:-cap]):
                        new_list.append(
                            mybir.InstNoOp(
                                name=f"{inst.name}-wsplit{kk}",
                                engine=inst.engine,
                                ins=[],
                                outs=[],
                                sync_info=mybir.SyncInfo(on_wait=[w], on_update=[]),
                            )
                        )
                    inst.sync_info = mybir.SyncInfo(
                        on_wait=waits[-cap:], on_update=list(si.on_update)
                    )
                new_list.append(inst)
            if changed:
                live = bb.instructions
                live.clear()
                for i in new_list:
                    bb.add_instruction(i)
    return nc


def build_gla(T=2048, groups=((0, 1, 2, 3), (4, 5, 6, 7)), n_devices=8,
              apply_beta=False, split_waits=True, use_silu=True):
    """Build the per-core SPMD program.  T = tokens per core."""
    assert T % 128 == 0
    NT = T // 128      # 128-token tiles
    KT = H // 128      # contraction slices

    nc = bass.Bass(num_devices=n_devices)
    xt_d = nc.declare_dram_parameter("xt", [H, T], BF16, isOutput=False)
    w_d = {
        n: nc.declare_dram_parameter(n, [H, H], BF16, isOutput=False)
        for n in ("wq", "wk", "wv", "wg", "wo")
    }
    beta_d = (
        nc.declare_dram_parameter("beta", [1, H], F32, isOutput=False)
        if apply_beta
        else None
    )
    out_d = nc.declare_dram_parameter("out", [T, H], F32, isOutput=True)

    ks_in = nc.dram_tensor("ks_in", [1, H], F32)
    ks_out = nc.dram_tensor("ks_out", [1, H], F32)

    def mm(ps, lhsT, rhs, start, stop):
        nc.tensor.matmul(ps, lhsT=lhsT, rhs=rhs, start=start, stop=stop)

    with tile.TileContext(nc) as tc:
        with (
            tc.tile_pool(name="singles", bufs=1) as singles,
            tc.tile_pool(name="w", bufs=4) as wpool,
            tc.tile_pool(name="xt", bufs=1) as xtpool,
            tc.tile_pool(name="kt", bufs=1) as ktpool,
            tc.tile_pool(name="qt", bufs=1) as qtpool,
            tc.tile_pool(name="elu", bufs=2) as elupool,
            tc.tile_pool(name="prod", bufs=2) as prodpool,
            tc.tile_pool(name="small", bufs=2) as smpool,
            tc.tile_pool(name="zu", bufs=2) as zupool,
            tc.tile_pool(name="ut", bufs=2) as utpool,
            tc.tile_pool(name="y", bufs=2) as ypool,
        ):
            ident = singles.tile([128, 128], BF16)
            make_identity(nc, ident)
            ones_col = singles.tile([128, 1], BF16)
            nc.vector.memset(ones_col, 1.0)
            qk_all = singles.tile([128, NT, NH], F32)
            # rsqrt bit-hack constants (as APs: immediate ints on uint ops
            # are unreliable through the f32 immediate path)
            c_shift1 = singles.tile([128, 1], U32)
            nc.vector.memset(c_shift1, 1)
            c_magic = singles.tile([128, 1], U32)
            nc.vector.memset(c_magic, 0x5F3759DF)

            xt_all = xtpool.tile([128, KT, T], BF16)
            kt_all = ktpool.tile([128, NT, H], BF16)
            qt_all = qtpool.tile([128, NT, H], BF16)

            def alloc_w():
                return wpool.tile([128, KT, H], BF16, tag="w", name="wslot")

            def load_w_slice(t, name, k):
                nc.sync.dma_start(out=t[:, k, :],
                                  in_=w_d[name][128 * k:128 * (k + 1), :])

            def elu1(dst, ps):
                # dst = elu(ps)+1 = min(exp(ps), 1) + relu(ps); ps is PSUM f32
                e = elupool.tile([128, 512], F32, tag="elue")
                r = elupool.tile([128, 512], F32, tag="elur")
                nc.scalar.activation(out=e, in_=ps, func=ACT_F.Exp)
                nc.scalar.activation(out=r, in_=ps, func=ACT_F.Relu)
                nc.vector.scalar_tensor_tensor(
                    out=dst, in0=e, scalar=1.0, in1=r,
                    op0=ALU.min, op1=ALU.add,
                )

            # interleave X^T with wk so the first matmul's inputs land early
            wk_t = alloc_w()
            for k in range(KT):
                nc.sync.dma_start(out=xt_all[:, k, :],
                                  in_=xt_d[128 * k:128 * (k + 1), :])
                load_w_slice(wk_t, "wk", k)
            wq_t = alloc_w()           # prefetched during phase 1a
            for k in range(KT):
                load_w_slice(wq_t, "wq", k)

            # -------- phase 1a: k projection + k_sum (k kept in SBUF) ------
            with (
                tc.tile_pool(name="ks", bufs=1, space="PSUM") as kspool,
                tc.tile_pool(name="pk", bufs=2, space="PSUM") as pkpool,
            ):
                ks_ps = kspool.tile([1, H], F32)

                def emit_ksum(t):
                    for n in range(2):
                        nc.tensor.matmul(
                            ks_ps[:, 512 * n:512 * (n + 1)],
                            lhsT=ones_col,
                            rhs=kt_all[:, t, 512 * n:512 * (n + 1)],
                            start=(t == 0 and n == 0),
                            stop=(t == NT - 1 and n == 1),
                        )

                for t in range(NT):
                    for n in range(2):
                        pk = pkpool.tile([128, 512], F32, tag="pk")
                        nsl = slice(512 * n, 512 * (n + 1))
                        for k in range(KT):
                            lhs = xt_all[:, k, 128 * t:128 * (t + 1)]
                            mm(pk, lhs, wk_t[:, k, nsl], k == 0, k == KT - 1)
                        elu1(kt_all[:, t, nsl], pk)
                    # ksum of the previous tile: its elu chain finished while
                    # this tile's matmuls ran, so the PE never waits on DVE
                    if t > 0:
                        emit_ksum(t - 1)
                emit_ksum(NT - 1)
                ks_sb = singles.tile([1, H], F32)
                nc.vector.tensor_copy(out=ks_sb, in_=ks_ps)
            # fire the AllReduce now -- phase 1b covers its latency
            nc.sync.dma_start(out=ks_in[:, :], in_=ks_sb)
            nc.gpsimd.collective_compute(
                "AllReduce", ALU.add,
                replica_groups=[list(g) for g in groups],
                ins=[ks_in[:, :]], outs=[ks_out[:, :]],
            )
            ksb_f32 = singles.tile([128, H], F32)
            nc.gpsimd.dma_start(out=ksb_f32,
                                in_=ks_out[0:1, :].to_broadcast([128, H]))
            ksb = singles.tile([128, H], BF16)
            nc.vector.tensor_copy(out=ksb, in_=ksb_f32)
            if apply_beta:
                beta_b = singles.tile([128, H], F32)
                nc.gpsimd.dma_start(out=beta_b,
                                    in_=beta_d[0:1, :].to_broadcast([128, H]))

            # -------- phase 1b: q projection + qk (q kept in SBUF) ---------
            wv_t = alloc_w()           # prefetched for phase 2
            wg_t = alloc_w()
            for k in range(KT):
                load_w_slice(wv_t, "wv", k)
                load_w_slice(wg_t, "wg", k)
            with tc.tile_pool(name="pq", bufs=2, space="PSUM") as pqpool:
                for t in range(NT):
                    for n in range(2):
                        pq = pqpool.tile([128, 512], F32, tag="pq")
                        nsl = slice(512 * n, 512 * (n + 1))
                        for k in range(KT):
                            lhs = xt_all[:, k, 128 * t:128 * (t + 1)]
                            mm(pq, lhs, wq_t[:, k, nsl], k == 0, k == KT - 1)
                        elu1(qt_all[:, t, nsl], pq)
                    prod = prodpool.tile([128, H], BF16, tag="prod")
                    nc.vector.tensor_mul(prod, qt_all[:, t, :], kt_all[:, t, :])
                    nc.vector.reduce_sum(
                        out=qk_all[:, t, :],
                        in_=prod.rearrange("p (h d) -> p h d", d=DK),
                        axis=AX.X,
                    )

            # ---------------- phase 2: v, g, z, LN, gate, Wo ----------------
            wo_t = alloc_w()           # rotates into wk's slot (dead)
            for k in range(KT):
                load_w_slice(wo_t, "wo", k)
            with (
                tc.tile_pool(name="pa", bufs=2, space="PSUM") as papool,
                tc.tile_pool(name="pb", bufs=2, space="PSUM") as pbpool,
                tc.tile_pool(name="pt", bufs=2, space="PSUM") as ptpool,
                tc.tile_pool(name="py", bufs=2, space="PSUM") as pypool,
            ):
                def back_end(u, rsig, t):
                    # transpose u into one bf16 PSUM bank, evacuate with a
                    # single scalar-engine copy, then the Wo matmuls; 1/sigma
                    # is folded into the PSUM->SBUF output move.
                    pt = ptpool.tile([128, H], BF16, tag="pt")
                    for k in range(KT):
                        nc.tensor.transpose(
                            pt[:, 128 * k:128 * (k + 1)],
                            u[:, 128 * k:128 * (k + 1)], ident)
                    ut = utpool.tile([128, H], BF16, tag="ut")
                    nc.scalar.copy(out=ut, in_=pt)
                    for n in range(2):
                        nsl = slice(512 * n, 512 * (n + 1))
                        py = pypool.tile([128, 512], F32, tag="py")
                        for k in range(KT):
                            mm(py, ut[:, 128 * k:128 * (k + 1)],
                               wo_t[:, k, nsl], k == 0, k == KT - 1)
                        y_sb = ypool.tile([128, 512], F32, tag="y")
                        if rsig is not None:
                            nc.vector.tensor_scalar(
                                out=y_sb, in0=py,
                                scalar1=rsig, scalar2=None, op0=ALU.mult,
                            )
                        else:
                            nc.vector.tensor_copy(out=y_sb, in_=py)
                        nc.sync.dma_start(
                            out=out_d[128 * t:128 * (t + 1), nsl], in_=y_sb)

                prev = None
                for t in range(NT):
                    s_t = zupool.tile([128, H], BF16, tag="s")
                    pvs = []
                    for n in range(2):
                        pv = papool.tile([128, 512], F32, tag="pa")
                        pg = pbpool.tile([128, 512], F32, tag="pb")
                        nsl = slice(512 * n, 512 * (n + 1))
                        for k in range(KT):
                            lhs = xt_all[:, k, 128 * t:128 * (t + 1)]
                            mm(pv, lhs, wv_t[:, k, nsl], k == 0, k == KT - 1)
                            mm(pg, lhs, wg_t[:, k, nsl], k == 0, k == KT - 1)
                        ssl = s_t[:, nsl]
                        if use_silu:
                            nc.scalar.activation(out=ssl, in_=pg, func=ACT_F.Silu)
                        else:  # CoreSim has no Silu table
                            nc.scalar.activation(out=ssl, in_=pg,
                                                 func=ACT_F.Sigmoid)
                            nc.vector.tensor_mul(ssl, ssl, pg)
                        pvs.append(pv)
                    # normalizer = per-head dot(q, k_sum)
                    nprod = prodpool.tile([128, H], BF16, tag="prod")
                    nc.vector.tensor_mul(nprod, qt_all[:, t, :], ksb)
                    norm = smpool.tile([128, NH], F32, tag="norm")
                    nc.vector.reduce_sum(
                        out=norm, in_=nprod.rearrange("p (h d) -> p h d", d=DK),
                        axis=AX.X,
                    )
                    rec = smpool.tile([128, NH], F32, tag="rec")
                    nc.vector.tensor_scalar_add(out=rec, in0=norm, scalar1=1e-6)
                    nc.vector.reciprocal(out=rec, in_=rec)
                    r = smpool.tile([128, NH], F32, tag="r")
                    nc.vector.tensor_mul(r, qk_all[:, t, :], rec)
                    # z = r (broadcast over d) * v
                    z = zupool.tile([128, H], BF16, tag="z")
                    for n in range(2):
                        rs = r[:, 8 * n:8 * (n + 1)]
                        r_b = bass.AP(tensor=rs.tensor, offset=rs.offset,
                                      ap=[list(rs.ap[0]), list(rs.ap[1]), [0, DK]])
                        nc.vector.tensor_tensor(
                            out=z[:, 512 * n:512 * (n + 1)],
                            in0=pvs[n], in1=r_b, op=ALU.mult,
                        )
                    # LayerNorm stats over the full 1024 features
                    st = smpool.tile([128, 2, nc.vector.BN_STATS_DIM], F32,
                                     tag="bnst")
                    for n in range(2):
                        nc.vector.bn_stats(out=st[:, n, :],
                                           in_=z[:, 512 * n:512 * (n + 1)])
                    mv = smpool.tile([128, nc.vector.BN_AGGR_DIM], F32, tag="mv")
                    nc.vector.bn_aggr(out=mv, in_=st)
                    # rsig = rsqrt(var + eps) on the DVE: exponent bit-hack
                    # seed + 2 Newton steps (max rel err ~5e-6).  Runs off the
                    # critical path; consumed only at Wo PSUM evacuation.
                    vq = smpool.tile([128, 1], F32, tag="vq")
                    nc.vector.tensor_scalar_add(out=vq, in0=mv[:, 1:2],
                                                scalar1=1e-5)
                    rsig = smpool.tile([128, 1], F32, tag="rsig")
                    nc.vector.tensor_scalar(
                        out=rsig.bitcast(U32), in0=vq.bitcast(U32),
                        scalar1=c_shift1[:, 0:1], scalar2=None,
                        op0=ALU.logical_shift_right,
                    )
                    nc.vector.tensor_tensor(
                        out=rsig.bitcast(U32), in0=c_magic,
                        in1=rsig.bitcast(U32), op=ALU.subtract,
                    )
                    nt1 = smpool.tile([128, 1], F32, tag="nt1")
                    for _ in range(2):
                        nc.vector.tensor_mul(nt1, rsig, rsig)
                        nc.vector.tensor_mul(nt1, nt1, vq)
                        nc.vector.tensor_scalar(
                            out=nt1, in0=nt1, scalar1=-0.5, scalar2=1.5,
                            op0=ALU.mult, op1=ALU.add,
                        )
                        nc.vector.tensor_mul(rsig, rsig, nt1)
                    # u = (z - mu) * silu(g); 1/sigma deferred past Wo
                    u = zupool.tile([128, H], BF16, tag="u")
                    if apply_beta:
                        # beta breaks the deferral: apply rsig here instead
                        nc.vector.tensor_scalar(
                            out=u, in0=z, scalar1=mv[:, 0:1], scalar2=rsig,
                            op0=ALU.subtract, op1=ALU.mult,
                        )
                        nc.vector.tensor_add(out=u, in0=u, in1=beta_b)
                        nc.vector.tensor_mul(u, u, s_t)
                        rsig_eff = None
                    else:
                        nc.vector.tensor_scalar(
                            out=u, in0=z, scalar1=mv[:, 0:1], scalar2=None,
                            op0=ALU.subtract,
                        )
                        nc.vector.tensor_mul(u, u, s_t)
                        rsig_eff = rsig
                    # software pipeline: run the previous tile's transposes +
                    # Wo matmuls while this tile's DVE chain executes
                    if prev is not None:
                        back_end(*prev)
                    prev = (u, rsig_eff, t)
                back_end(*prev)
    return _split_multi_waits(nc) if split_waits else nc


# ------------------------------------------------------------------
# host glue
# ------------------------------------------------------------------
_CACHE = {}
LAST_RESULT = None


def kernel(hidden_states, Wq, Wk, Wv, Wg, Wo, gamma, beta):
    import ml_dtypes
    bf16 = ml_dtypes.bfloat16

    hs = np.asarray(hidden_states, dtype=np.float32)
    Wq = np.asarray(Wq, dtype=np.float32)
    Wk = np.asarray(Wk, dtype=np.float32)
    Wv = np.asarray(Wv, dtype=np.float32)
    Wg = np.asarray(Wg, dtype=np.float32)
    Wo = np.asarray(Wo, dtype=np.float32)
    gamma = np.asarray(gamma, dtype=np.float32)
    beta = np.asarray(beta, dtype=np.float32)

    b, s, h = hs.shape
    tokens = hs.reshape(b * s, h)
    n_tok = b * s
    T = n_tok // N_CORES
    assert s % T == 0, "core token shards must not straddle batches"
    cores_per_batch = s // T

    groups = tuple(
        tuple(range(bi * cores_per_batch, (bi + 1) * cores_per_batch))
        for bi in range(b)
    )
    apply_beta = bool(np.any(beta))

    key = (T, groups, apply_beta)
    if key not in _CACHE:
        _CACHE[key] = build_gla(T=T, groups=groups, apply_beta=apply_beta)
    nc = _CACHE[key]

    wo_eff = (gamma[:, None] * Wo).astype(bf16)
    wq_b = Wq.astype(bf16)
    wk_b = Wk.astype(bf16)
    wv_b = Wv.astype(bf16)
    wg_b = Wg.astype(bf16)
    in_maps = []
    for i in range(N_CORES):
        m = {
            "xt": np.ascontiguousarray(tokens[i * T:(i + 1) * T].T).astype(bf16),
            "wq": wq_b, "wk": wk_b, "wv": wv_b, "wg": wg_b, "wo": wo_eff,
        }
        if apply_beta:
            m["beta"] = beta.reshape(1, h)
        in_maps.append(m)

    res = run_bass_kernel_spmd(
        nc, in_maps, core_ids=list(range(N_CORES)),
        trace=bool(os.environ.get("GLA_TRACE")),
    )
    global LAST_RESULT
    LAST_RESULT = res
    out = np.concatenate([res.results[i]["out"] for i in range(N_CORES)], axis=0)
    return out.reshape(b, s, h)


# revision 16
# speedup vs baseline: 1.2427x; 1.0278x over previous
"""Gated linear attention kernel for one TRN2 chip (8 NeuronCores).

Math (see reference):
    q = elu(X Wq)+1, k = elu(X Wk)+1, v = X Wv, g = X Wg
    qk = sum_d(q*k) per head; k_sum = sum_seq(k); norm = sum_d(q*k_sum)
    z = qk*v/(norm+1e-6); z = LayerNorm(z)*gamma+beta; out = (z*silu(g)) Wo

Sharding: data-parallel over the 16384 tokens, 2048 per core; cores 0-3 own
batch 0, cores 4-7 batch 1.  The only cross-core coupling is k_sum (a [1,1024]
vector per batch) -> AllReduce within 4-core groups, overlapped with the
q-projection phase.

Key layout decisions (v2, tuned against the perfetto trace of v1):
  * Everything the PE touches is bf16: X^T, the five weights, k, q, u.
    bf16 streams at the same 1 col/cycle as float32r but LDWEIGHTS is 2x
    faster (FWL) and DMA/SBUF cost halves.  PSUM accumulation stays fp32.
  * X^T (4 MB), k (4 MB) and q (4 MB) are SBUF-resident for the whole
    kernel -- no DRAM spills, X is DMA'd exactly once.  Total HBM traffic
    is 22 MB/core (was 84 MB), so the PE never waits on DMA.
  * elu(x)+1 == min(exp(x),1) + relu(x) exactly (2 ACT + 1 DVE op); exp and
    relu live in the same ACT table so phase 1 has no table reloads.
  * The LayerNorm rsqrt is computed on the DVE (Newton iteration seeded by
    the exponent bit-hack) and -- because 1/sigma is a per-token scalar that
    commutes with the Wo matmul -- applied to the Wo *output* during PSUM
    evacuation.  Phase 2's scalar engine runs only Silu+Copy (one table),
    eliminating the per-tile Silu<->Sqrt table thrash (42 us in v1).
  * Phase 2 is software-pipelined one tile deep: the transposes + Wo
    matmuls of tile t-1 are enqueued between the v/g matmuls of tile t so
    the PE never drains while the DVE normalization chain runs.
  * The 8 u-transposes of a tile write disjoint 128-col slices of ONE bf16
    PSUM bank, evacuated by a single scalar-engine copy.
gamma is folded into Wo on the host; beta==0 is verified on the host (the
slower beta path is only built when beta is nonzero).
"""

import os

import numpy as np

import concourse.bass as bass
import concourse.mybir as mybir
import concourse.tile as tile
from concourse.bass_utils import run_bass_kernel_spmd
from concourse.masks import make_identity

F32 = mybir.dt.float32
BF16 = mybir.dt.bfloat16
U32 = mybir.dt.uint32
AX = mybir.AxisListType
ALU = mybir.AluOpType
ACT_F = mybir.ActivationFunctionType

H = 1024
NH = 16
DK = 64
N_CORES = 8


def _split_multi_waits(nc, cap=1):
    """walrus in this image rejects instructions with more than ~2 sync waits
    (Tile attaches several to its kernel-tail drain).  Move excess waits onto
    preceding same-engine NoOps."""
    for f in nc.m.functions:
        for bb in f.blocks:
            insts = bb.instructions
            new_list = []
            changed = False
            for inst in insts:
                si = inst.sync_info
                waits = list(si.on_wait) if si else []
                if len(waits) > cap:
                    changed = True
                    for kk, w in enumerate(waits[:-cap]):
                        new_list.append(
                            mybir.InstNoOp(
                                name=f"{inst.name}-wsplit{kk}",
                                engine=inst.engine,
                                ins=[],
                                outs=[],
                                sync_info=mybir.SyncInfo(on_wait=[w], on_update=[]),
                            )
                        )
                    inst.sync_info = mybir.SyncInfo(
                        on_wait=waits[-cap:], on_update=list(si.on_update)
                    )
                new_list.append(inst)
            if changed:
                live = bb.instructions
                live.clear()
                for i in new_list:
                    bb.add_instruction(i)
    return nc


def build_gla(T=2048, groups=((0, 1, 2, 3), (4, 5, 6, 7)), n_devices=8,
              apply_beta=False, split_waits=True, use_silu=True):
    """Build the per-core SPMD program.  T = tokens per core."""
    assert T % 128 == 0
    NT = T // 128      # 128-token tiles
    KT = H // 128      # contraction slices

    nc = bass.Bass(num_devices=n_devices)
    xt_d = nc.declare_dram_parameter("xt", [H, T], BF16, isOutput=False)
    w_d = {
        n: nc.declare_dram_parameter(n, [H, H], BF16, isOutput=False)
        for n in ("wq", "wk", "wv", "wg", "wo")
    }
    beta_d = (
        nc.declare_dram_parameter("beta", [1, H], F32, isOutput=False)
        if apply_beta
        else None
    )
    out_d = nc.declare_dram_parameter("out", [T, H], F32, isOutput=True)

    ks_in = nc.dram_tensor("ks_in", [1, H], F32)
    ks_out = nc.dram_tensor("ks_out", [1, H], F32)

    def mm(ps, lhsT, rhs, start, stop):
        nc.tensor.matmul(ps, lhsT=lhsT, rhs=rhs, start=start, stop=stop)

    with tile.TileContext(nc) as tc:
        with (
            tc.tile_pool(name="singles", bufs=1) as singles,
            tc.tile_pool(name="w", bufs=4) as wpool,
            tc.tile_pool(name="xt", bufs=1) as xtpool,
            tc.tile_pool(name="kt", bufs=1) as ktpool,
            tc.tile_pool(name="qt", bufs=1) as qtpool,
            tc.tile_pool(name="elu", bufs=2) as elupool,
            tc.tile_pool(name="prod", bufs=1) as prodpool,
            tc.tile_pool(name="small", bufs=3) as smpool,
            tc.tile_pool(name="z2", bufs=2) as zpool,
            tc.tile_pool(name="su", bufs=3) as supool,
            tc.tile_pool(name="ut", bufs=2) as utpool,
            tc.tile_pool(name="y", bufs=2) as ypool,
        ):
            ident = singles.tile([128, 128], BF16)
            make_identity(nc, ident)
            ones_col = singles.tile([128, 1], BF16)
            nc.vector.memset(ones_col, 1.0)
            qk_all = singles.tile([128, NT, NH], F32)
            # rsqrt bit-hack constants (as APs: immediate ints on uint ops
            # are unreliable through the f32 immediate path)
            c_shift1 = singles.tile([128, 1], U32)
            nc.vector.memset(c_shift1, 1)
            c_magic = singles.tile([128, 1], U32)
            nc.vector.memset(c_magic, 0x5F3759DF)

            xt_all = xtpool.tile([128, KT, T], BF16)
            kt_all = ktpool.tile([128, NT, H], BF16)
            qt_all = qtpool.tile([128, NT, H], BF16)

            def alloc_w():
                return wpool.tile([128, KT, H], BF16, tag="w", name="wslot")

            def load_w_slice(t, name, k):
                nc.sync.dma_start(out=t[:, k, :],
                                  in_=w_d[name][128 * k:128 * (k + 1), :])

            def elu1(dst, ps):
                # dst = elu(ps)+1 = min(exp(ps), 1) + relu(ps); ps is PSUM f32
                e = elupool.tile([128, 512], F32, tag="elue")
                r = elupool.tile([128, 512], F32, tag="elur")
                nc.scalar.activation(out=e, in_=ps, func=ACT_F.Exp)
                nc.scalar.activation(out=r, in_=ps, func=ACT_F.Relu)
                nc.vector.scalar_tensor_tensor(
                    out=dst, in0=e, scalar=1.0, in1=r,
                    op0=ALU.min, op1=ALU.add,
                )

            # interleave X^T with wk so the first matmul's inputs land early
            wk_t = alloc_w()
            for k in range(KT):
                nc.sync.dma_start(out=xt_all[:, k, :],
                                  in_=xt_d[128 * k:128 * (k + 1), :])
                load_w_slice(wk_t, "wk", k)
            wq_t = alloc_w()           # prefetched during phase 1a
            for k in range(KT):
                load_w_slice(wq_t, "wq", k)

            # -------- phase 1a: k projection + k_sum (k kept in SBUF) ------
            with (
                tc.tile_pool(name="ks", bufs=1, space="PSUM") as kspool,
                tc.tile_pool(name="pk", bufs=2, space="PSUM") as pkpool,
            ):
                ks_ps = kspool.tile([1, H], F32)

                def emit_ksum(t):
                    for n in range(2):
                        nc.tensor.matmul(
                            ks_ps[:, 512 * n:512 * (n + 1)],
                            lhsT=ones_col,
                            rhs=kt_all[:, t, 512 * n:512 * (n + 1)],
                            start=(t == 0 and n == 0),
                            stop=(t == NT - 1 and n == 1),
                        )

                for t in range(NT):
                    for n in range(2):
                        pk = pkpool.tile([128, 512], F32, tag="pk")
                        nsl = slice(512 * n, 512 * (n + 1))
                        for k in range(KT):
                            lhs = xt_all[:, k, 128 * t:128 * (t + 1)]
                            mm(pk, lhs, wk_t[:, k, nsl], k == 0, k == KT - 1)
                        elu1(kt_all[:, t, nsl], pk)
                    # ksum of the previous tile: its elu chain finished while
                    # this tile's matmuls ran, so the PE never waits on DVE
                    if t > 0:
                        emit_ksum(t - 1)
                emit_ksum(NT - 1)
                # the AllReduce chain runs under high_priority: the Tile
                # scheduler otherwise parks the ks_sb copy ~30us deep into
                # phase 1b's vector work, which delays the collective enough
                # to stall the 1b->2 phase boundary by ~9us.
                with tc.high_priority():
                    ks_sb = singles.tile([1, H], F32)
                    nc.vector.tensor_copy(out=ks_sb, in_=ks_ps)
            with tc.high_priority():
                nc.sync.dma_start(out=ks_in[:, :], in_=ks_sb)
                nc.gpsimd.collective_compute(
                    "AllReduce", ALU.add,
                    replica_groups=[list(g) for g in groups],
                    ins=[ks_in[:, :]], outs=[ks_out[:, :]],
                )
                ksb_f32 = singles.tile([128, H], F32)
                nc.gpsimd.dma_start(out=ksb_f32,
                                    in_=ks_out[0:1, :].to_broadcast([128, H]))
                ksb = singles.tile([128, H], BF16)
                nc.gpsimd.tensor_copy(out=ksb, in_=ksb_f32)
            if apply_beta:
                beta_b = singles.tile([128, H], F32)
                nc.gpsimd.dma_start(out=beta_b,
                                    in_=beta_d[0:1, :].to_broadcast([128, H]))

            # -------- phase 1b: q projection + qk (q kept in SBUF) ---------
            wv_t = alloc_w()           # prefetched for phase 2
            wg_t = alloc_w()
            for k in range(KT):
                load_w_slice(wv_t, "wv", k)
                load_w_slice(wg_t, "wg", k)
            with tc.tile_pool(name="pq", bufs=2, space="PSUM") as pqpool:
                for t in range(NT):
                    for n in range(2):
                        pq = pqpool.tile([128, 512], F32, tag="pq")
                        nsl = slice(512 * n, 512 * (n + 1))
                        for k in range(KT):
                            lhs = xt_all[:, k, 128 * t:128 * (t + 1)]
                            mm(pq, lhs, wq_t[:, k, nsl], k == 0, k == KT - 1)
                        elu1(qt_all[:, t, nsl], pq)
                    prod = prodpool.tile([128, H], BF16, tag="prod")
                    nc.vector.tensor_mul(prod, qt_all[:, t, :], kt_all[:, t, :])
                    nc.vector.reduce_sum(
                        out=qk_all[:, t, :],
                        in_=prod.rearrange("p (h d) -> p h d", d=DK),
                        axis=AX.X,
                    )

            # ---------------- phase 2: v, g, z, LN, gate, Wo ----------------
            wo_t = alloc_w()           # rotates into wk's slot (dead)
            for k in range(KT):
                load_w_slice(wo_t, "wo", k)
            with (
                tc.tile_pool(name="pa", bufs=2, space="PSUM") as papool,
                tc.tile_pool(name="pb", bufs=2, space="PSUM") as pbpool,
                tc.tile_pool(name="pt", bufs=2, space="PSUM") as ptpool,
                tc.tile_pool(name="py", bufs=2, space="PSUM") as pypool,
            ):
                def back_end(u, rsig, t):
                    # transpose u into one bf16 PSUM bank, evacuate with a
                    # single scalar-engine copy, then the Wo matmuls; 1/sigma
                    # is folded into the PSUM->SBUF output move.
                    pt = ptpool.tile([128, H], BF16, tag="pt")
                    for k in range(KT):
                        nc.tensor.transpose(
                            pt[:, 128 * k:128 * (k + 1)],
                            u[:, 128 * k:128 * (k + 1)], ident)
                    ut = utpool.tile([128, H], BF16, tag="ut")
                    nc.scalar.copy(out=ut, in_=pt)
                    for n in range(2):
                        nsl = slice(512 * n, 512 * (n + 1))
                        py = pypool.tile([128, 512], F32, tag="py")
                        for k in range(KT):
                            mm(py, ut[:, 128 * k:128 * (k + 1)],
                               wo_t[:, k, nsl], k == 0, k == KT - 1)
                        y_sb = ypool.tile([128, 512], F32, tag="y")
                        if rsig is not None:
                            nc.vector.tensor_scalar(
                                out=y_sb, in0=py,
                                scalar1=rsig, scalar2=None, op0=ALU.mult,
                            )
                        else:
                            nc.vector.tensor_copy(out=y_sb, in_=py)
                        nc.sync.dma_start(
                            out=out_d[128 * t:128 * (t + 1), nsl], in_=y_sb)

                # 2-deep software pipeline: run tile t-2's transposes + Wo
                # while tiles t-1/t's DVE chains execute, so Vector-queue
                # scheduling jitter can never stall the PE.
                prevs = []
                for t in range(NT):
                    s_t = supool.tile([128, H], BF16, tag="s")
                    pvs = []
                    for n in range(2):
                        pv = papool.tile([128, 512], F32, tag="pa")
                        pg = pbpool.tile([128, 512], F32, tag="pb")
                        nsl = slice(512 * n, 512 * (n + 1))
                        for k in range(KT):
                            lhs = xt_all[:, k, 128 * t:128 * (t + 1)]
                            mm(pv, lhs, wv_t[:, k, nsl], k == 0, k == KT - 1)
                            mm(pg, lhs, wg_t[:, k, nsl], k == 0, k == KT - 1)
                        ssl = s_t[:, nsl]
                        if use_silu:
                            nc.scalar.activation(out=ssl, in_=pg, func=ACT_F.Silu)
                        else:  # CoreSim has no Silu table
                            nc.scalar.activation(out=ssl, in_=pg,
                                                 func=ACT_F.Sigmoid)
                            nc.vector.tensor_mul(ssl, ssl, pg)
                        pvs.append(pv)
                    # normalizer = per-head dot(q, k_sum)
                    nprod = prodpool.tile([128, H], BF16, tag="prod")
                    nc.vector.tensor_mul(nprod, qt_all[:, t, :], ksb)
                    norm = smpool.tile([128, NH], F32, tag="norm")
                    nc.vector.reduce_sum(
                        out=norm, in_=nprod.rearrange("p (h d) -> p h d", d=DK),
                        axis=AX.X,
                    )
                    rec = smpool.tile([128, NH], F32, tag="rec")
                    nc.vector.tensor_scalar_add(out=rec, in0=norm, scalar1=1e-6)
                    nc.vector.reciprocal(out=rec, in_=rec)
                    r = smpool.tile([128, NH], F32, tag="r")
                    nc.vector.tensor_mul(r, qk_all[:, t, :], rec)
                    # z = r (broadcast over d) * v
                    z = zpool.tile([128, H], BF16, tag="z")
                    for n in range(2):
                        rs = r[:, 8 * n:8 * (n + 1)]
                        r_b = bass.AP(tensor=rs.tensor, offset=rs.offset,
                                      ap=[list(rs.ap[0]), list(rs.ap[1]), [0, DK]])
                        nc.vector.tensor_tensor(
                            out=z[:, 512 * n:512 * (n + 1)],
                            in0=pvs[n], in1=r_b, op=ALU.mult,
                        )
                    # LayerNorm stats over the full 1024 features
                    st = smpool.tile([128, 2, nc.vector.BN_STATS_DIM], F32,
                                     tag="bnst")
                    for n in range(2):
                        nc.vector.bn_stats(out=st[:, n, :],
                                           in_=z[:, 512 * n:512 * (n + 1)])
                    mv = smpool.tile([128, nc.vector.BN_AGGR_DIM], F32, tag="mv")
                    nc.vector.bn_aggr(out=mv, in_=st)
                    # rsig = rsqrt(var + eps) on the DVE: exponent bit-hack
                    # seed + 2 Newton steps (max rel err ~5e-6).  Runs off the
                    # critical path; consumed only at Wo PSUM evacuation.
                    vq = smpool.tile([128, 1], F32, tag="vq")
                    nc.vector.tensor_scalar_add(out=vq, in0=mv[:, 1:2],
                                                scalar1=1e-5)
                    rsig = smpool.tile([128, 1], F32, tag="rsig")
                    nc.vector.tensor_scalar(
                        out=rsig.bitcast(U32), in0=vq.bitcast(U32),
                        scalar1=c_shift1[:, 0:1], scalar2=None,
                        op0=ALU.logical_shift_right,
                    )
                    nc.vector.tensor_tensor(
                        out=rsig.bitcast(U32), in0=c_magic,
                        in1=rsig.bitcast(U32), op=ALU.subtract,
                    )
                    nt1 = smpool.tile([128, 1], F32, tag="nt1")
                    for _ in range(2):
                        nc.vector.tensor_mul(nt1, rsig, rsig)
                        nc.vector.tensor_mul(nt1, nt1, vq)
                        nc.vector.tensor_scalar(
                            out=nt1, in0=nt1, scalar1=-0.5, scalar2=1.5,
                            op0=ALU.mult, op1=ALU.add,
                        )
                        nc.vector.tensor_mul(rsig, rsig, nt1)
                    # u = (z - mu) * silu(g); 1/sigma deferred past Wo
                    u = supool.tile([128, H], BF16, tag="u")
                    if apply_beta:
                        # beta breaks the deferral: apply rsig here instead
                        nc.vector.tensor_scalar(
                            out=u, in0=z, scalar1=mv[:, 0:1], scalar2=rsig,
                            op0=ALU.subtract, op1=ALU.mult,
                        )
                        nc.vector.tensor_add(out=u, in0=u, in1=beta_b)
                        nc.vector.tensor_mul(u, u, s_t)
                        rsig_eff = None
                    else:
                        nc.vector.tensor_scalar(
                            out=u, in0=z, scalar1=mv[:, 0:1], scalar2=None,
                            op0=ALU.subtract,
                        )
                        nc.vector.tensor_mul(u, u, s_t)
                        rsig_eff = rsig
                    prevs.append((u, rsig_eff, t))
                    if len(prevs) > 2:
                        back_end(*prevs.pop(0))
                for p in prevs:
                    back_end(*p)
    return _split_multi_waits(nc) if split_waits else nc


# ------------------------------------------------------------------
# host glue
# ------------------------------------------------------------------
_CACHE = {}
LAST_RESULT = None


def kernel(hidden_states, Wq, Wk, Wv, Wg, Wo, gamma, beta):
    import ml_dtypes
    bf16 = ml_dtypes.bfloat16

    hs = np.asarray(hidden_states, dtype=np.float32)
    Wq = np.asarray(Wq, dtype=np.float32)
    Wk = np.asarray(Wk, dtype=np.float32)
    Wv = np.asarray(Wv, dtype=np.float32)
    Wg = np.asarray(Wg, dtype=np.float32)
    Wo = np.asarray(Wo, dtype=np.float32)
    gamma = np.asarray(gamma, dtype=np.float32)
    beta = np.asarray(beta, dtype=np.float32)

    b, s, h = hs.shape
    tokens = hs.reshape(b * s, h)
    n_tok = b * s
    T = n_tok // N_CORES
    assert s % T == 0, "core token shards must not straddle batches"
    cores_per_batch = s // T

    groups = tuple(
        tuple(range(bi * cores_per_batch, (bi + 1) * cores_per_batch))
        for bi in range(b)
    )
    apply_beta = bool(np.any(beta))

    key = (T, groups, apply_beta)
    if key not in _CACHE:
        _CACHE[key] = build_gla(T=T, groups=groups, apply_beta=apply_beta)
    nc = _CACHE[key]

    wo_eff = (gamma[:, None] * Wo).astype(bf16)
    wq_b = Wq.astype(bf16)
    wk_b = Wk.astype(bf16)
    wv_b = Wv.astype(bf16)
    wg_b = Wg.astype(bf16)
    in_maps = []
    for i in range(N_CORES):
        m = {
            "xt": np.ascontiguousarray(tokens[i * T:(i + 1) * T].T).astype(bf16),
            "wq": wq_b, "wk": wk_b, "wv": wv_b, "wg": wg_b, "wo": wo_eff,
        }
        if apply_beta:
            m["beta"] = beta.reshape(1, h)
        in_maps.append(m)

    res = run_bass_kernel_spmd(
        nc, in_maps, core_ids=list(range(N_CORES)),
        trace=bool(os.environ.get("GLA_TRACE")),
    )
    global LAST_RESULT
    LAST_RESULT = res
    out = np.concatenate([res.results[i]["out"] for i in range(N_CORES)], axis=0)
    return out.reshape(b, s, h)


# revision 18
# speedup vs baseline: 1.2633x; 1.0166x over previous
"""Gated linear attention kernel for one TRN2 chip (8 NeuronCores).

Math (see reference):
    q = elu(X Wq)+1, k = elu(X Wk)+1, v = X Wv, g = X Wg
    qk = sum_d(q*k) per head; k_sum = sum_seq(k); norm = sum_d(q*k_sum)
    z = qk*v/(norm+1e-6); z = LayerNorm(z)*gamma+beta; out = (z*silu(g)) Wo

Sharding: data-parallel over the 16384 tokens, 2048 per core; cores 0-3 own
batch 0, cores 4-7 batch 1.  The only cross-core coupling is k_sum (a [1,1024]
vector per batch) -> AllReduce within 4-core groups, overlapped with the
q-projection phase.

Key layout decisions (v2, tuned against the perfetto trace of v1):
  * Everything the PE touches is bf16: X^T, the five weights, k, q, u.
    bf16 streams at the same 1 col/cycle as float32r but LDWEIGHTS is 2x
    faster (FWL) and DMA/SBUF cost halves.  PSUM accumulation stays fp32.
  * X^T (4 MB), k (4 MB) and q (4 MB) are SBUF-resident for the whole
    kernel -- no DRAM spills, X is DMA'd exactly once.  Total HBM traffic
    is 22 MB/core (was 84 MB), so the PE never waits on DMA.
  * elu(x)+1 == min(exp(x),1) + relu(x) exactly (2 ACT + 1 DVE op); exp and
    relu live in the same ACT table so phase 1 has no table reloads.
  * The LayerNorm rsqrt is computed on the DVE (Newton iteration seeded by
    the exponent bit-hack) and -- because 1/sigma is a per-token scalar that
    commutes with the Wo matmul -- applied to the Wo *output* during PSUM
    evacuation.  Phase 2's scalar engine runs only Silu+Copy (one table),
    eliminating the per-tile Silu<->Sqrt table thrash (42 us in v1).
  * Phase 2 is software-pipelined one tile deep: the transposes + Wo
    matmuls of tile t-1 are enqueued between the v/g matmuls of tile t so
    the PE never drains while the DVE normalization chain runs.
  * The 8 u-transposes of a tile write disjoint 128-col slices of ONE bf16
    PSUM bank, evacuated by a single scalar-engine copy.
gamma is folded into Wo on the host; beta==0 is verified on the host (the
slower beta path is only built when beta is nonzero).
"""

import os

import numpy as np

import concourse.bass as bass
import concourse.mybir as mybir
import concourse.tile as tile
from concourse.bass_utils import run_bass_kernel_spmd
from concourse.masks import make_identity

F32 = mybir.dt.float32
BF16 = mybir.dt.bfloat16
U32 = mybir.dt.uint32
AX = mybir.AxisListType
ALU = mybir.AluOpType
ACT_F = mybir.ActivationFunctionType

H = 1024
NH = 16
DK = 64
N_CORES = 8


def _split_multi_waits(nc, cap=1):
    """walrus in this image rejects instructions with more than ~2 sync waits
    (Tile attaches several to its kernel-tail drain).  Move excess waits onto
    preceding same-engine NoOps."""
    for f in nc.m.functions:
        for bb in f.blocks:
            insts = bb.instructions
            new_list = []
            changed = False
            for inst in insts:
                si = inst.sync_info
                waits = list(si.on_wait) if si else []
                if len(waits) > cap:
                    changed = True
                    for kk, w in enumerate(waits[:-cap]):
                        new_list.append(
                            mybir.InstNoOp(
                                name=f"{inst.name}-wsplit{kk}",
                                engine=inst.engine,
                                ins=[],
                                outs=[],
                                sync_info=mybir.SyncInfo(on_wait=[w], on_update=[]),
                            )
                        )
                    inst.sync_info = mybir.SyncInfo(
                        on_wait=waits[-cap:], on_update=list(si.on_update)
                    )
                new_list.append(inst)
            if changed:
                live = bb.instructions
                live.clear()
                for i in new_list:
                    bb.add_instruction(i)
    return nc


def build_gla(T=2048, groups=((0, 1, 2, 3), (4, 5, 6, 7)), n_devices=8,
              apply_beta=False, split_waits=True, use_silu=True):
    """Build the per-core SPMD program.  T = tokens per core."""
    assert T % 128 == 0
    NT = T // 128      # 128-token tiles
    KT = H // 128      # contraction slices

    nc = bass.Bass(num_devices=n_devices)
    xt_d = nc.declare_dram_parameter("xt", [H, T], BF16, isOutput=False)
    w_d = {
        n: nc.declare_dram_parameter(n, [H, H], BF16, isOutput=False)
        for n in ("wq", "wk", "wv", "wg", "wo")
    }
    beta_d = (
        nc.declare_dram_parameter("beta", [1, H], F32, isOutput=False)
        if apply_beta
        else None
    )
    out_d = nc.declare_dram_parameter("out", [T, H], F32, isOutput=True)

    ks_in = nc.dram_tensor("ks_in", [1, H], F32)
    ks_out = nc.dram_tensor("ks_out", [1, H], F32)

    def mm(ps, lhsT, rhs, start, stop):
        nc.tensor.matmul(ps, lhsT=lhsT, rhs=rhs, start=start, stop=stop)

    with tile.TileContext(nc) as tc:
        with (
            tc.tile_pool(name="singles", bufs=1) as singles,
            tc.tile_pool(name="w", bufs=4) as wpool,
            tc.tile_pool(name="xt", bufs=1) as xtpool,
            tc.tile_pool(name="kt", bufs=1) as ktpool,
            tc.tile_pool(name="qt", bufs=1) as qtpool,
            tc.tile_pool(name="elu", bufs=2) as elupool,
            tc.tile_pool(name="prod", bufs=1) as prodpool,
            tc.tile_pool(name="small", bufs=3) as smpool,
            tc.tile_pool(name="z2", bufs=2) as zpool,
            tc.tile_pool(name="su", bufs=3) as supool,
            tc.tile_pool(name="ut", bufs=2) as utpool,
            tc.tile_pool(name="y", bufs=2) as ypool,
        ):
            ident = singles.tile([128, 128], BF16)
            make_identity(nc, ident)
            ones_col = singles.tile([128, 1], BF16)
            nc.vector.memset(ones_col, 1.0)
            qk_all = singles.tile([128, NT, NH], F32)
            # rsqrt bit-hack constants (as APs: immediate ints on uint ops
            # are unreliable through the f32 immediate path)
            c_shift1 = singles.tile([128, 1], U32)
            nc.vector.memset(c_shift1, 1)
            c_magic = singles.tile([128, 1], U32)
            nc.vector.memset(c_magic, 0x5F3759DF)

            xt_all = xtpool.tile([128, KT, T], BF16)
            kt_all = ktpool.tile([128, NT, H], BF16)
            qt_all = qtpool.tile([128, NT, H], BF16)

            def alloc_w():
                return wpool.tile([128, KT, H], BF16, tag="w", name="wslot")

            def load_w_slice(t, name, k):
                nc.sync.dma_start(out=t[:, k, :],
                                  in_=w_d[name][128 * k:128 * (k + 1), :])

            def elu1(dst, ps):
                # dst = elu(ps)+1 = min(exp(ps), 1) + relu(ps); ps is PSUM f32
                e = elupool.tile([128, 512], F32, tag="elue")
                r = elupool.tile([128, 512], F32, tag="elur")
                nc.scalar.activation(out=e, in_=ps, func=ACT_F.Exp)
                nc.scalar.activation(out=r, in_=ps, func=ACT_F.Relu)
                nc.vector.scalar_tensor_tensor(
                    out=dst, in0=e, scalar=1.0, in1=r,
                    op0=ALU.min, op1=ALU.add,
                )

            # stage the initial loads by column block so tile 0's matmuls can
            # start after ~2 MB instead of the full 8 MB X+Wk+Wq burst
            wk_t = alloc_w()
            for k in range(KT):
                ksl = slice(128 * k, 128 * (k + 1))
                nc.sync.dma_start(out=xt_all[:, k, 0:512],
                                  in_=xt_d[ksl, 0:512])
                nc.sync.dma_start(out=wk_t[:, k, 0:512],
                                  in_=w_d["wk"][ksl, 0:512])
            for k in range(KT):
                ksl = slice(128 * k, 128 * (k + 1))
                nc.sync.dma_start(out=wk_t[:, k, 512:1024],
                                  in_=w_d["wk"][ksl, 512:1024])
                nc.sync.dma_start(out=xt_all[:, k, 512:1024],
                                  in_=xt_d[ksl, 512:1024])
            for k in range(KT):
                ksl = slice(128 * k, 128 * (k + 1))
                nc.sync.dma_start(out=xt_all[:, k, 1024:T],
                                  in_=xt_d[ksl, 1024:T])
            wq_t = alloc_w()           # prefetched during phase 1a
            for k in range(KT):
                load_w_slice(wq_t, "wq", k)

            # -------- phase 1a: k projection + k_sum (k kept in SBUF) ------
            with (
                tc.tile_pool(name="ks", bufs=1, space="PSUM") as kspool,
                tc.tile_pool(name="pk", bufs=2, space="PSUM") as pkpool,
            ):
                ks_ps = kspool.tile([1, H], F32)

                def emit_ksum(t):
                    for n in range(2):
                        nc.tensor.matmul(
                            ks_ps[:, 512 * n:512 * (n + 1)],
                            lhsT=ones_col,
                            rhs=kt_all[:, t, 512 * n:512 * (n + 1)],
                            start=(t == 0 and n == 0),
                            stop=(t == NT - 1 and n == 1),
                        )

                for t in range(NT):
                    for n in range(2):
                        pk = pkpool.tile([128, 512], F32, tag="pk")
                        nsl = slice(512 * n, 512 * (n + 1))
                        for k in range(KT):
                            lhs = xt_all[:, k, 128 * t:128 * (t + 1)]
                            mm(pk, lhs, wk_t[:, k, nsl], k == 0, k == KT - 1)
                        elu1(kt_all[:, t, nsl], pk)
                    # ksum of the previous tile: its elu chain finished while
                    # this tile's matmuls ran, so the PE never waits on DVE
                    if t > 0:
                        emit_ksum(t - 1)
                emit_ksum(NT - 1)
                # the AllReduce chain runs under high_priority: the Tile
                # scheduler otherwise parks the ks_sb copy ~30us deep into
                # phase 1b's vector work, which delays the collective enough
                # to stall the 1b->2 phase boundary by ~9us.
                with tc.high_priority():
                    ks_sb = singles.tile([1, H], F32)
                    nc.vector.tensor_copy(out=ks_sb, in_=ks_ps)
            with tc.high_priority():
                nc.sync.dma_start(out=ks_in[:, :], in_=ks_sb)
                nc.gpsimd.collective_compute(
                    "AllReduce", ALU.add,
                    replica_groups=[list(g) for g in groups],
                    ins=[ks_in[:, :]], outs=[ks_out[:, :]],
                )
                ksb_f32 = singles.tile([128, H], F32)
                nc.gpsimd.dma_start(out=ksb_f32,
                                    in_=ks_out[0:1, :].to_broadcast([128, H]))
                ksb = singles.tile([128, H], BF16)
                nc.gpsimd.tensor_copy(out=ksb, in_=ksb_f32)
            if apply_beta:
                beta_b = singles.tile([128, H], F32)
                nc.gpsimd.dma_start(out=beta_b,
                                    in_=beta_d[0:1, :].to_broadcast([128, H]))

            # -------- phase 1b: q projection + qk (q kept in SBUF) ---------
            wv_t = alloc_w()           # prefetched for phase 2
            wg_t = alloc_w()
            for k in range(KT):
                load_w_slice(wv_t, "wv", k)
                load_w_slice(wg_t, "wg", k)
            with tc.tile_pool(name="pq", bufs=2, space="PSUM") as pqpool:
                for t in range(NT):
                    for n in range(2):
                        pq = pqpool.tile([128, 512], F32, tag="pq")
                        nsl = slice(512 * n, 512 * (n + 1))
                        for k in range(KT):
                            lhs = xt_all[:, k, 128 * t:128 * (t + 1)]
                            mm(pq, lhs, wq_t[:, k, nsl], k == 0, k == KT - 1)
                        elu1(qt_all[:, t, nsl], pq)
                    prod = prodpool.tile([128, H], BF16, tag="prod")
                    nc.vector.tensor_mul(prod, qt_all[:, t, :], kt_all[:, t, :])
                    nc.vector.reduce_sum(
                        out=qk_all[:, t, :],
                        in_=prod.rearrange("p (h d) -> p h d", d=DK),
                        axis=AX.X,
                    )

            # ---------------- phase 2: v, g, z, LN, gate, Wo ----------------
            wo_t = alloc_w()           # rotates into wk's slot (dead)
            for k in range(KT):
                load_w_slice(wo_t, "wo", k)
            # pool creation order controls PSUM bank placement: pt/py (first
            # needed ~25us into phase 2) take the banks recycled from 1b's pq
            # pool, so the v/g matmuls (pa/pb) start on long-free banks and
            # don't wait for tile-15's elu chain to release pq.
            with (
                tc.tile_pool(name="pt", bufs=2, space="PSUM") as ptpool,
                tc.tile_pool(name="py", bufs=2, space="PSUM") as pypool,
                tc.tile_pool(name="pa", bufs=2, space="PSUM") as papool,
                tc.tile_pool(name="pb", bufs=2, space="PSUM") as pbpool,
            ):
                def back_end(u, rsig, t):
                    # transpose u into one bf16 PSUM bank, evacuate with a
                    # single scalar-engine copy, then the Wo matmuls; 1/sigma
                    # is folded into the PSUM->SBUF output move.
                    pt = ptpool.tile([128, H], BF16, tag="pt")
                    for k in range(KT):
                        nc.tensor.transpose(
                            pt[:, 128 * k:128 * (k + 1)],
                            u[:, 128 * k:128 * (k + 1)], ident)
                    ut = utpool.tile([128, H], BF16, tag="ut")
                    nc.scalar.copy(out=ut, in_=pt)
                    for n in range(2):
                        nsl = slice(512 * n, 512 * (n + 1))
                        py = pypool.tile([128, 512], F32, tag="py")
                        for k in range(KT):
                            mm(py, ut[:, 128 * k:128 * (k + 1)],
                               wo_t[:, k, nsl], k == 0, k == KT - 1)
                        y_sb = ypool.tile([128, 512], F32, tag="y")
                        if rsig is not None:
                            nc.vector.tensor_scalar(
                                out=y_sb, in0=py,
                                scalar1=rsig, scalar2=None, op0=ALU.mult,
                            )
                        else:
                            nc.vector.tensor_copy(out=y_sb, in_=py)
                        nc.sync.dma_start(
                            out=out_d[128 * t:128 * (t + 1), nsl], in_=y_sb)

                # 2-deep software pipeline: run tile t-2's transposes + Wo
                # while tiles t-1/t's DVE chains execute, so Vector-queue
                # scheduling jitter can never stall the PE.
                prevs = []
                for t in range(NT):
                    s_t = supool.tile([128, H], BF16, tag="s")
                    pvs = []
                    for n in range(2):
                        pv = papool.tile([128, 512], F32, tag="pa")
                        pg = pbpool.tile([128, 512], F32, tag="pb")
                        nsl = slice(512 * n, 512 * (n + 1))
                        for k in range(KT):
                            lhs = xt_all[:, k, 128 * t:128 * (t + 1)]
                            mm(pv, lhs, wv_t[:, k, nsl], k == 0, k == KT - 1)
                            mm(pg, lhs, wg_t[:, k, nsl], k == 0, k == KT - 1)
                        ssl = s_t[:, nsl]
                        if use_silu:
                            nc.scalar.activation(out=ssl, in_=pg, func=ACT_F.Silu)
                        else:  # CoreSim has no Silu table
                            nc.scalar.activation(out=ssl, in_=pg,
                                                 func=ACT_F.Sigmoid)
                            nc.vector.tensor_mul(ssl, ssl, pg)
                        pvs.append(pv)
                    # normalizer = per-head dot(q, k_sum)
                    nprod = prodpool.tile([128, H], BF16, tag="prod")
                    nc.vector.tensor_mul(nprod, qt_all[:, t, :], ksb)
                    norm = smpool.tile([128, NH], F32, tag="norm")
                    nc.vector.reduce_sum(
                        out=norm, in_=nprod.rearrange("p (h d) -> p h d", d=DK),
                        axis=AX.X,
                    )
                    rec = smpool.tile([128, NH], F32, tag="rec")
                    nc.vector.tensor_scalar_add(out=rec, in0=norm, scalar1=1e-6)
                    nc.vector.reciprocal(out=rec, in_=rec)
                    r = smpool.tile([128, NH], F32, tag="r")
                    nc.vector.tensor_mul(r, qk_all[:, t, :], rec)
                    # z = r (broadcast over d) * v
                    z = zpool.tile([128, H], BF16, tag="z")
                    for n in range(2):
                        rs = r[:, 8 * n:8 * (n + 1)]
                        r_b = bass.AP(tensor=rs.tensor, offset=rs.offset,
                                      ap=[list(rs.ap[0]), list(rs.ap[1]), [0, DK]])
                        nc.vector.tensor_tensor(
                            out=z[:, 512 * n:512 * (n + 1)],
                            in0=pvs[n], in1=r_b, op=ALU.mult,
                        )
                    # LayerNorm stats over the full 1024 features
                    st = smpool.tile([128, 2, nc.vector.BN_STATS_DIM], F32,
                                     tag="bnst")
                    for n in range(2):
                        nc.vector.bn_stats(out=st[:, n, :],
                                           in_=z[:, 512 * n:512 * (n + 1)])
                    mv = smpool.tile([128, nc.vector.BN_AGGR_DIM], F32, tag="mv")
                    nc.vector.bn_aggr(out=mv, in_=st)
                    # rsig = rsqrt(var + eps) on the DVE: exponent bit-hack
                    # seed + 2 Newton steps (max rel err ~5e-6).  Runs off the
                    # critical path; consumed only at Wo PSUM evacuation.
                    vq = smpool.tile([128, 1], F32, tag="vq")
                    nc.vector.tensor_scalar_add(out=vq, in0=mv[:, 1:2],
                                                scalar1=1e-5)
                    rsig = smpool.tile([128, 1], F32, tag="rsig")
                    nc.vector.tensor_scalar(
                        out=rsig.bitcast(U32), in0=vq.bitcast(U32),
                        scalar1=c_shift1[:, 0:1], scalar2=None,
                        op0=ALU.logical_shift_right,
                    )
                    nc.vector.tensor_tensor(
                        out=rsig.bitcast(U32), in0=c_magic,
                        in1=rsig.bitcast(U32), op=ALU.subtract,
                    )
                    nt1 = smpool.tile([128, 1], F32, tag="nt1")
                    for _ in range(2):
                        nc.vector.tensor_mul(nt1, rsig, rsig)
                        nc.vector.tensor_mul(nt1, nt1, vq)
                        nc.vector.tensor_scalar(
                            out=nt1, in0=nt1, scalar1=-0.5, scalar2=1.5,
                            op0=ALU.mult, op1=ALU.add,
                        )
                        nc.vector.tensor_mul(rsig, rsig, nt1)
                    # u = (z - mu) * silu(g); 1/sigma deferred past Wo
                    u = supool.tile([128, H], BF16, tag="u")
                    if apply_beta:
                        # beta breaks the deferral: apply rsig here instead
                        nc.vector.tensor_scalar(
                            out=u, in0=z, scalar1=mv[:, 0:1], scalar2=rsig,
                            op0=ALU.subtract, op1=ALU.mult,
                        )
                        nc.vector.tensor_add(out=u, in0=u, in1=beta_b)
                        nc.vector.tensor_mul(u, u, s_t)
                        rsig_eff = None
                    else:
                        nc.vector.tensor_scalar(
                            out=u, in0=z, scalar1=mv[:, 0:1], scalar2=None,
                            op0=ALU.subtract,
                        )
                        nc.vector.tensor_mul(u, u, s_t)
                        rsig_eff = rsig
                    prevs.append((u, rsig_eff, t))
                    if len(prevs) > 2:
                        back_end(*prevs.pop(0))
                for p in prevs:
                    back_end(*p)
    return _split_multi_waits(nc) if split_waits else nc


# ------------------------------------------------------------------
# host glue
# ------------------------------------------------------------------
_CACHE = {}
LAST_RESULT = None


def kernel(hidden_states, Wq, Wk, Wv, Wg, Wo, gamma, beta):
    import ml_dtypes
    bf16 = ml_dtypes.bfloat16

    hs = np.asarray(hidden_states, dtype=np.float32)
    Wq = np.asarray(Wq, dtype=np.float32)
    Wk = np.asarray(Wk, dtype=np.float32)
    Wv = np.asarray(Wv, dtype=np.float32)
    Wg = np.asarray(Wg, dtype=np.float32)
    Wo = np.asarray(Wo, dtype=np.float32)
    gamma = np.asarray(gamma, dtype=np.float32)
    beta = np.asarray(beta, dtype=np.float32)

    b, s, h = hs.shape
    tokens = hs.reshape(b * s, h)
    n_tok = b * s
    T = n_tok // N_CORES
    assert s % T == 0, "core token shards must not straddle batches"
    cores_per_batch = s // T

    groups = tuple(
        tuple(range(bi * cores_per_batch, (bi + 1) * cores_per_batch))
        for bi in range(b)
    )
    apply_beta = bool(np.any(beta))

    key = (T, groups, apply_beta)
    if key not in _CACHE:
        _CACHE[key] = build_gla(T=T, groups=groups, apply_beta=apply_beta)
    nc = _CACHE[key]

    wo_eff = (gamma[:, None] * Wo).astype(bf16)
    wq_b = Wq.astype(bf16)
    wk_b = Wk.astype(bf16)
    wv_b = Wv.astype(bf16)
    wg_b = Wg.astype(bf16)
    in_maps = []
    for i in range(N_CORES):
        m = {
            "xt": np.ascontiguousarray(tokens[i * T:(i + 1) * T].T).astype(bf16),
            "wq": wq_b, "wk": wk_b, "wv": wv_b, "wg": wg_b, "wo": wo_eff,
        }
        if apply_beta:
            m["beta"] = beta.reshape(1, h)
        in_maps.append(m)

    res = run_bass_kernel_spmd(
        nc, in_maps, core_ids=list(range(N_CORES)),
        trace=bool(os.environ.get("GLA_TRACE")),
    )
    global LAST_RESULT
    LAST_RESULT = res
    out = np.concatenate([res.results[i]["out"] for i in range(N_CORES)], axis=0)
    return out.reshape(b, s, h)


# revision 21
# speedup vs baseline: 1.3024x; 1.0309x over previous
"""Gated linear attention kernel for one TRN2 chip (8 NeuronCores).

Math (see reference):
    q = elu(X Wq)+1, k = elu(X Wk)+1, v = X Wv, g = X Wg
    qk = sum_d(q*k) per head; k_sum = sum_seq(k); norm = sum_d(q*k_sum)
    z = qk*v/(norm+1e-6); z = LayerNorm(z)*gamma+beta; out = (z*silu(g)) Wo

Sharding: data-parallel over the 16384 tokens, 2048 per core; cores 0-3 own
batch 0, cores 4-7 batch 1.  The only cross-core coupling is k_sum (a [1,1024]
vector per batch) -> AllReduce within 4-core groups, overlapped with the
q-projection phase.

Key layout decisions (v2, tuned against the perfetto trace of v1):
  * Everything the PE touches is bf16: X^T, the five weights, k, q, u.
    bf16 streams at the same 1 col/cycle as float32r but LDWEIGHTS is 2x
    faster (FWL) and DMA/SBUF cost halves.  PSUM accumulation stays fp32.
  * X^T (4 MB), k (4 MB) and q (4 MB) are SBUF-resident for the whole
    kernel -- no DRAM spills, X is DMA'd exactly once.  Total HBM traffic
    is 22 MB/core (was 84 MB), so the PE never waits on DMA.
  * elu(x)+1 == min(exp(x),1) + relu(x) exactly (2 ACT + 1 DVE op); exp and
    relu live in the same ACT table so phase 1 has no table reloads.
  * The LayerNorm rsqrt is computed on the DVE (Newton iteration seeded by
    the exponent bit-hack) and -- because 1/sigma is a per-token scalar that
    commutes with the Wo matmul -- applied to the Wo *output* during PSUM
    evacuation.  Phase 2's scalar engine runs only Silu+Copy (one table),
    eliminating the per-tile Silu<->Sqrt table thrash (42 us in v1).
  * Phase 2 is software-pipelined one tile deep: the transposes + Wo
    matmuls of tile t-1 are enqueued between the v/g matmuls of tile t so
    the PE never drains while the DVE normalization chain runs.
  * The 8 u-transposes of a tile write disjoint 128-col slices of ONE bf16
    PSUM bank, evacuated by a single scalar-engine copy.
gamma is folded into Wo on the host; beta==0 is verified on the host (the
slower beta path is only built when beta is nonzero).
"""

import os

import numpy as np

import concourse.bass as bass
import concourse.mybir as mybir
import concourse.tile as tile
from concourse.bass_utils import run_bass_kernel_spmd
from concourse.masks import make_identity

F32 = mybir.dt.float32
BF16 = mybir.dt.bfloat16
U32 = mybir.dt.uint32
AX = mybir.AxisListType
ALU = mybir.AluOpType
ACT_F = mybir.ActivationFunctionType

H = 1024
NH = 16
DK = 64
N_CORES = 8


def _split_multi_waits(nc, cap=1):
    """walrus in this image rejects instructions with more than ~2 sync waits
    (Tile attaches several to its kernel-tail drain).  Move excess waits onto
    preceding same-engine NoOps."""
    for f in nc.m.functions:
        for bb in f.blocks:
            insts = bb.instructions
            new_list = []
            changed = False
            for inst in insts:
                si = inst.sync_info
                waits = list(si.on_wait) if si else []
                if len(waits) > cap:
                    changed = True
                    for kk, w in enumerate(waits[:-cap]):
                        new_list.append(
                            mybir.InstNoOp(
                                name=f"{inst.name}-wsplit{kk}",
                                engine=inst.engine,
                                ins=[],
                                outs=[],
                                sync_info=mybir.SyncInfo(on_wait=[w], on_update=[]),
                            )
                        )
                    inst.sync_info = mybir.SyncInfo(
                        on_wait=waits[-cap:], on_update=list(si.on_update)
                    )
                new_list.append(inst)
            if changed:
                live = bb.instructions
                live.clear()
                for i in new_list:
                    bb.add_instruction(i)
    return nc


def build_gla(T=2048, groups=((0, 1, 2, 3), (4, 5, 6, 7)), n_devices=8,
              apply_beta=False, split_waits=True, use_silu=True):
    """Build the per-core SPMD program.  T = tokens per core."""
    assert T % 128 == 0
    NT = T // 128      # 128-token tiles
    KT = H // 128      # contraction slices

    nc = bass.Bass(num_devices=n_devices)
    xt_d = nc.declare_dram_parameter("xt", [H, T], BF16, isOutput=False)
    w_d = {
        n: nc.declare_dram_parameter(n, [H, H], BF16, isOutput=False)
        for n in ("wq", "wk", "wv", "wg", "wo")
    }
    beta_d = (
        nc.declare_dram_parameter("beta", [1, H], F32, isOutput=False)
        if apply_beta
        else None
    )
    out_d = nc.declare_dram_parameter("out", [T, H], F32, isOutput=True)

    ks_in = nc.dram_tensor("ks_in", [1, H], F32)
    ks_out = nc.dram_tensor("ks_out", [1, H], F32)

    def mm(ps, lhsT, rhs, start, stop):
        nc.tensor.matmul(ps, lhsT=lhsT, rhs=rhs, start=start, stop=stop)

    with tile.TileContext(nc) as tc:
        with (
            tc.tile_pool(name="singles", bufs=1) as singles,
            tc.tile_pool(name="w", bufs=4) as wpool,
            tc.tile_pool(name="xt", bufs=1) as xtpool,
            tc.tile_pool(name="kt", bufs=1) as ktpool,
            tc.tile_pool(name="qt", bufs=1) as qtpool,
            tc.tile_pool(name="elu", bufs=2) as elupool,
            tc.tile_pool(name="prod", bufs=1) as prodpool,
            tc.tile_pool(name="small", bufs=3) as smpool,
            tc.tile_pool(name="z2", bufs=2) as zpool,
            tc.tile_pool(name="su", bufs=3) as supool,
            tc.tile_pool(name="ut", bufs=2) as utpool,
            tc.tile_pool(name="y", bufs=2) as ypool,
        ):
            ones_col = singles.tile([128, 1], BF16)
            nc.vector.memset(ones_col, 1.0)
            qk_all = singles.tile([128, NT, NH], F32)
            # rsqrt bit-hack constants (as APs: immediate ints on uint ops
            # are unreliable through the f32 immediate path)
            c_shift1 = singles.tile([128, 1], U32)
            nc.vector.memset(c_shift1, 1)
            c_magic = singles.tile([128, 1], U32)
            nc.vector.memset(c_magic, 0x5F3759DF)

            xt_all = xtpool.tile([128, KT, T], BF16)
            kt_all = ktpool.tile([128, NT, H], BF16)
            qt_all = qtpool.tile([128, NT, H], BF16)

            def alloc_w():
                return wpool.tile([128, KT, H], BF16, tag="w", name="wslot")

            def load_w_slice(t, name, k):
                nc.sync.dma_start(out=t[:, k, :],
                                  in_=w_d[name][128 * k:128 * (k + 1), :])

            def elu1(dst, ps):
                # dst = elu(ps)+1 = min(exp(ps), 1) + relu(ps); ps is PSUM f32
                e = elupool.tile([128, 512], F32, tag="elue")
                r = elupool.tile([128, 512], F32, tag="elur")
                nc.scalar.activation(out=e, in_=ps, func=ACT_F.Exp)
                nc.scalar.activation(out=r, in_=ps, func=ACT_F.Relu)
                nc.vector.scalar_tensor_tensor(
                    out=dst, in0=e, scalar=1.0, in1=r,
                    op0=ALU.min, op1=ALU.add,
                )

            # stage the initial loads by column block so tile 0's matmuls can
            # start after ~2 MB instead of the full 8 MB X+Wk+Wq burst
            wk_t = alloc_w()
            for k in range(KT):
                ksl = slice(128 * k, 128 * (k + 1))
                nc.sync.dma_start(out=xt_all[:, k, 0:512],
                                  in_=xt_d[ksl, 0:512])
                nc.sync.dma_start(out=wk_t[:, k, 0:512],
                                  in_=w_d["wk"][ksl, 0:512])
            for k in range(KT):
                ksl = slice(128 * k, 128 * (k + 1))
                nc.sync.dma_start(out=wk_t[:, k, 512:1024],
                                  in_=w_d["wk"][ksl, 512:1024])
                nc.sync.dma_start(out=xt_all[:, k, 512:1024],
                                  in_=xt_d[ksl, 512:1024])
            for k in range(KT):
                ksl = slice(128 * k, 128 * (k + 1))
                nc.sync.dma_start(out=xt_all[:, k, 1024:T],
                                  in_=xt_d[ksl, 1024:T])
            wq_t = alloc_w()           # prefetched during phase 1a
            for k in range(KT):
                load_w_slice(wq_t, "wq", k)

            # -------- phase 1a: k projection + k_sum (k kept in SBUF) ------
            with (
                tc.tile_pool(name="ks", bufs=1, space="PSUM") as kspool,
                tc.tile_pool(name="pk", bufs=2, space="PSUM") as pkpool,
            ):
                ks_ps = kspool.tile([1, H], F32)

                def emit_ksum(t):
                    for n in range(2):
                        nc.tensor.matmul(
                            ks_ps[:, 512 * n:512 * (n + 1)],
                            lhsT=ones_col,
                            rhs=kt_all[:, t, 512 * n:512 * (n + 1)],
                            start=(t == 0 and n == 0),
                            stop=(t == NT - 1 and n == 1),
                        )

                for t in range(NT):
                    for n in range(2):
                        pk = pkpool.tile([128, 512], F32, tag="pk")
                        nsl = slice(512 * n, 512 * (n + 1))
                        for k in range(KT):
                            lhs = xt_all[:, k, 128 * t:128 * (t + 1)]
                            mm(pk, lhs, wk_t[:, k, nsl], k == 0, k == KT - 1)
                        elu1(kt_all[:, t, nsl], pk)
                    # ksum of the previous tile: its elu chain finished while
                    # this tile's matmuls ran, so the PE never waits on DVE
                    if t > 0:
                        emit_ksum(t - 1)
                emit_ksum(NT - 1)
                # the AllReduce chain runs under high_priority: the Tile
                # scheduler otherwise parks the ks_sb copy ~30us deep into
                # phase 1b's vector work, which delays the collective enough
                # to stall the 1b->2 phase boundary by ~9us.
                with tc.high_priority():
                    ks_sb = singles.tile([1, H], F32)
                    nc.vector.tensor_copy(out=ks_sb, in_=ks_ps)
            with tc.high_priority():
                nc.sync.dma_start(out=ks_in[:, :], in_=ks_sb)
                nc.gpsimd.collective_compute(
                    "AllReduce", ALU.add,
                    replica_groups=[list(g) for g in groups],
                    ins=[ks_in[:, :]], outs=[ks_out[:, :]],
                )
                ksb_f32 = singles.tile([128, H], F32)
                nc.gpsimd.dma_start(out=ksb_f32,
                                    in_=ks_out[0:1, :].to_broadcast([128, H]))
                ksb = singles.tile([128, H], BF16)
                nc.gpsimd.tensor_copy(out=ksb, in_=ksb_f32)
            if apply_beta:
                beta_b = singles.tile([128, H], F32)
                nc.gpsimd.dma_start(out=beta_b,
                                    in_=beta_d[0:1, :].to_broadcast([128, H]))

            # -------- phase 1b: q projection + qk (q kept in SBUF) ---------
            wv_t = alloc_w()           # prefetched for phase 2
            wg_t = alloc_w()
            for k in range(KT):
                load_w_slice(wv_t, "wv", k)
                load_w_slice(wg_t, "wg", k)
            with tc.tile_pool(name="pq", bufs=2, space="PSUM") as pqpool:
                for t in range(NT):
                    for n in range(2):
                        pq = pqpool.tile([128, 512], F32, tag="pq")
                        nsl = slice(512 * n, 512 * (n + 1))
                        for k in range(KT):
                            lhs = xt_all[:, k, 128 * t:128 * (t + 1)]
                            mm(pq, lhs, wq_t[:, k, nsl], k == 0, k == KT - 1)
                        elu1(qt_all[:, t, nsl], pq)
                    prod = prodpool.tile([128, H], BF16, tag="prod")
                    nc.vector.tensor_mul(prod, qt_all[:, t, :], kt_all[:, t, :])
                    nc.vector.reduce_sum(
                        out=qk_all[:, t, :],
                        in_=prod.rearrange("p (h d) -> p h d", d=DK),
                        axis=AX.X,
                    )

            # ---------------- phase 2: v, g, z, LN, gate, Wo ----------------
            wo_t = alloc_w()           # rotates into wk's slot (dead)
            for k in range(KT):
                load_w_slice(wo_t, "wo", k)
            # pool creation order controls PSUM bank placement: py (first
            # needed ~25us into phase 2) takes the banks recycled from 1b's
            # pq pool, so the v/g matmuls (pa/pb) start on long-free banks
            # and don't wait for tile-15's elu chain to release pq.
            with (
                tc.tile_pool(name="py", bufs=2, space="PSUM") as pypool,
                tc.tile_pool(name="pa", bufs=3, space="PSUM") as papool,
                tc.tile_pool(name="pb", bufs=3, space="PSUM") as pbpool,
            ):
                def back_end(u, rsig, t):
                    # u^T via the DMA XBAR hardware transpose (2-byte dtypes
                    # only; verified layout ut[p,k,:] == u[:,128k+p]) -- keeps
                    # the 128 transposes/tile off the PE; 1/sigma is folded
                    # into the Wo PSUM->SBUF output move.
                    ut = utpool.tile([128, KT, 128], BF16, tag="ut")
                    nc.sync.dma_start_transpose(ut, u)
                    for n in range(2):
                        nsl = slice(512 * n, 512 * (n + 1))
                        py = pypool.tile([128, 512], F32, tag="py")
                        for k in range(KT):
                            mm(py, ut[:, k, :],
                               wo_t[:, k, nsl], k == 0, k == KT - 1)
                        y_sb = ypool.tile([128, 512], F32, tag="y")
                        if rsig is not None:
                            nc.vector.tensor_scalar(
                                out=y_sb, in0=py,
                                scalar1=rsig, scalar2=None, op0=ALU.mult,
                            )
                        else:
                            nc.vector.tensor_copy(out=y_sb, in_=py)
                        nc.sync.dma_start(
                            out=out_d[128 * t:128 * (t + 1), nsl], in_=y_sb)

                # 2-deep software pipeline: run tile t-2's transposes + Wo
                # while tiles t-1/t's DVE chains execute, so Vector-queue
                # scheduling jitter can never stall the PE.
                prevs = []
                for t in range(NT):
                    s_t = supool.tile([128, H], BF16, tag="s")
                    pvs = []
                    for n in range(2):
                        pv = papool.tile([128, 512], F32, tag="pa")
                        pg = pbpool.tile([128, 512], F32, tag="pb")
                        nsl = slice(512 * n, 512 * (n + 1))
                        for k in range(KT):
                            lhs = xt_all[:, k, 128 * t:128 * (t + 1)]
                            mm(pv, lhs, wv_t[:, k, nsl], k == 0, k == KT - 1)
                            mm(pg, lhs, wg_t[:, k, nsl], k == 0, k == KT - 1)
                        ssl = s_t[:, nsl]
                        if use_silu:
                            nc.scalar.activation(out=ssl, in_=pg, func=ACT_F.Silu)
                        else:  # CoreSim has no Silu table
                            nc.scalar.activation(out=ssl, in_=pg,
                                                 func=ACT_F.Sigmoid)
                            nc.vector.tensor_mul(ssl, ssl, pg)
                        pvs.append(pv)
                    # normalizer = per-head dot(q, k_sum)
                    nprod = prodpool.tile([128, H], BF16, tag="prod")
                    nc.vector.tensor_mul(nprod, qt_all[:, t, :], ksb)
                    norm = smpool.tile([128, NH], F32, tag="norm")
                    nc.vector.reduce_sum(
                        out=norm, in_=nprod.rearrange("p (h d) -> p h d", d=DK),
                        axis=AX.X,
                    )
                    rec = smpool.tile([128, NH], F32, tag="rec")
                    nc.vector.tensor_scalar_add(out=rec, in0=norm, scalar1=1e-6)
                    nc.vector.reciprocal(out=rec, in_=rec)
                    r = smpool.tile([128, NH], F32, tag="r")
                    nc.vector.tensor_mul(r, qk_all[:, t, :], rec)
                    # z = r (broadcast over d) * v
                    z = zpool.tile([128, H], BF16, tag="z")
                    for n in range(2):
                        rs = r[:, 8 * n:8 * (n + 1)]
                        r_b = bass.AP(tensor=rs.tensor, offset=rs.offset,
                                      ap=[list(rs.ap[0]), list(rs.ap[1]), [0, DK]])
                        nc.vector.tensor_tensor(
                            out=z[:, 512 * n:512 * (n + 1)],
                            in0=pvs[n], in1=r_b, op=ALU.mult,
                        )
                    # LayerNorm stats over the full 1024 features
                    st = smpool.tile([128, 2, nc.vector.BN_STATS_DIM], F32,
                                     tag="bnst")
                    for n in range(2):
                        nc.vector.bn_stats(out=st[:, n, :],
                                           in_=z[:, 512 * n:512 * (n + 1)])
                    mv = smpool.tile([128, nc.vector.BN_AGGR_DIM], F32, tag="mv")
                    nc.vector.bn_aggr(out=mv, in_=st)
                    # rsig = rsqrt(var + eps) on the DVE: exponent bit-hack
                    # seed + 2 Newton steps (max rel err ~5e-6).  Runs off the
                    # critical path; consumed only at Wo PSUM evacuation.
                    vq = smpool.tile([128, 1], F32, tag="vq")
                    nc.vector.tensor_scalar_add(out=vq, in0=mv[:, 1:2],
                                                scalar1=1e-5)
                    rsig = smpool.tile([128, 1], F32, tag="rsig")
                    nc.vector.tensor_scalar(
                        out=rsig.bitcast(U32), in0=vq.bitcast(U32),
                        scalar1=c_shift1[:, 0:1], scalar2=None,
                        op0=ALU.logical_shift_right,
                    )
                    nc.vector.tensor_tensor(
                        out=rsig.bitcast(U32), in0=c_magic,
                        in1=rsig.bitcast(U32), op=ALU.subtract,
                    )
                    nt1 = smpool.tile([128, 1], F32, tag="nt1")
                    for _ in range(2):
                        nc.vector.tensor_mul(nt1, rsig, rsig)
                        nc.vector.tensor_mul(nt1, nt1, vq)
                        nc.vector.tensor_scalar(
                            out=nt1, in0=nt1, scalar1=-0.5, scalar2=1.5,
                            op0=ALU.mult, op1=ALU.add,
                        )
                        nc.vector.tensor_mul(rsig, rsig, nt1)
                    # u = (z - mu) * silu(g); 1/sigma deferred past Wo
                    u = supool.tile([128, H], BF16, tag="u")
                    if apply_beta:
                        # beta breaks the deferral: apply rsig here instead
                        nc.vector.tensor_scalar(
                            out=u, in0=z, scalar1=mv[:, 0:1], scalar2=rsig,
                            op0=ALU.subtract, op1=ALU.mult,
                        )
                        nc.vector.tensor_add(out=u, in0=u, in1=beta_b)
                        nc.vector.tensor_mul(u, u, s_t)
                        rsig_eff = None
                    else:
                        nc.vector.tensor_scalar(
                            out=u, in0=z, scalar1=mv[:, 0:1], scalar2=None,
                            op0=ALU.subtract,
                        )
                        nc.vector.tensor_mul(u, u, s_t)
                        rsig_eff = rsig
                    prevs.append((u, rsig_eff, t))
                    if len(prevs) > 2:
                        back_end(*prevs.pop(0))
                for p in prevs:
                    back_end(*p)
    return _split_multi_waits(nc) if split_waits else nc


# ------------------------------------------------------------------
# host glue
# ------------------------------------------------------------------
_CACHE = {}
LAST_RESULT = None


def kernel(hidden_states, Wq, Wk, Wv, Wg, Wo, gamma, beta):
    import ml_dtypes
    bf16 = ml_dtypes.bfloat16

    hs = np.asarray(hidden_states, dtype=np.float32)
    Wq = np.asarray(Wq, dtype=np.float32)
    Wk = np.asarray(Wk, dtype=np.float32)
    Wv = np.asarray(Wv, dtype=np.float32)
    Wg = np.asarray(Wg, dtype=np.float32)
    Wo = np.asarray(Wo, dtype=np.float32)
    gamma = np.asarray(gamma, dtype=np.float32)
    beta = np.asarray(beta, dtype=np.float32)

    b, s, h = hs.shape
    tokens = hs.reshape(b * s, h)
    n_tok = b * s
    T = n_tok // N_CORES
    assert s % T == 0, "core token shards must not straddle batches"
    cores_per_batch = s // T

    groups = tuple(
        tuple(range(bi * cores_per_batch, (bi + 1) * cores_per_batch))
        for bi in range(b)
    )
    apply_beta = bool(np.any(beta))

    key = (T, groups, apply_beta)
    if key not in _CACHE:
        _CACHE[key] = build_gla(T=T, groups=groups, apply_beta=apply_beta)
    nc = _CACHE[key]

    wo_eff = (gamma[:, None] * Wo).astype(bf16)
    wq_b = Wq.astype(bf16)
    wk_b = Wk.astype(bf16)
    wv_b = Wv.astype(bf16)
    wg_b = Wg.astype(bf16)
    in_maps = []
    for i in range(N_CORES):
        m = {
            "xt": np.ascontiguousarray(tokens[i * T:(i + 1) * T].T).astype(bf16),
            "wq": wq_b, "wk": wk_b, "wv": wv_b, "wg": wg_b, "wo": wo_eff,
        }
        if apply_beta:
            m["beta"] = beta.reshape(1, h)
        in_maps.append(m)

    res = run_bass_kernel_spmd(
        nc, in_maps, core_ids=list(range(N_CORES)),
        trace=bool(os.environ.get("GLA_TRACE")),
    )
    global LAST_RESULT
    LAST_RESULT = res
    out = np.concatenate([res.results[i]["out"] for i in range(N_CORES)], axis=0)
    return out.reshape(b, s, h)


# revision 27
# speedup vs baseline: 1.3034x; 1.0008x over previous
"""Gated linear attention kernel for one TRN2 chip (8 NeuronCores).

Math (see reference):
    q = elu(X Wq)+1, k = elu(X Wk)+1, v = X Wv, g = X Wg
    qk = sum_d(q*k) per head; k_sum = sum_seq(k); norm = sum_d(q*k_sum)
    z = qk*v/(norm+1e-6); z = LayerNorm(z)*gamma+beta; out = (z*silu(g)) Wo

Sharding: data-parallel over the 16384 tokens, 2048 per core; cores 0-3 own
batch 0, cores 4-7 batch 1.  The only cross-core coupling is k_sum (a [1,1024]
vector per batch) -> AllReduce within 4-core groups, overlapped with the
q-projection phase.

Key layout decisions (v2, tuned against the perfetto trace of v1):
  * Everything the PE touches is bf16: X^T, the five weights, k, q, u.
    bf16 streams at the same 1 col/cycle as float32r but LDWEIGHTS is 2x
    faster (FWL) and DMA/SBUF cost halves.  PSUM accumulation stays fp32.
  * X^T (4 MB), k (4 MB) and q (4 MB) are SBUF-resident for the whole
    kernel -- no DRAM spills, X is DMA'd exactly once.  Total HBM traffic
    is 22 MB/core (was 84 MB), so the PE never waits on DMA.
  * elu(x)+1 == min(exp(x),1) + relu(x) exactly (2 ACT + 1 DVE op); exp and
    relu live in the same ACT table so phase 1 has no table reloads.
  * The LayerNorm rsqrt is computed on the DVE (Newton iteration seeded by
    the exponent bit-hack) and -- because 1/sigma is a per-token scalar that
    commutes with the Wo matmul -- applied to the Wo *output* during PSUM
    evacuation.  Phase 2's scalar engine runs only Silu+Copy (one table),
    eliminating the per-tile Silu<->Sqrt table thrash (42 us in v1).
  * Phase 2 is software-pipelined one tile deep: the transposes + Wo
    matmuls of tile t-1 are enqueued between the v/g matmuls of tile t so
    the PE never drains while the DVE normalization chain runs.
  * The 8 u-transposes of a tile write disjoint 128-col slices of ONE bf16
    PSUM bank, evacuated by a single scalar-engine copy.
gamma is folded into Wo on the host; beta==0 is verified on the host (the
slower beta path is only built when beta is nonzero).
"""

import os

import numpy as np

import concourse.bass as bass
import concourse.mybir as mybir
import concourse.tile as tile
from concourse.bass_utils import run_bass_kernel_spmd
from concourse.masks import make_identity

F32 = mybir.dt.float32
BF16 = mybir.dt.bfloat16
U32 = mybir.dt.uint32
AX = mybir.AxisListType
ALU = mybir.AluOpType
ACT_F = mybir.ActivationFunctionType

H = 1024
NH = 16
DK = 64
N_CORES = 8


def _split_multi_waits(nc, cap=1):
    """walrus in this image rejects instructions with more than ~2 sync waits
    (Tile attaches several to its kernel-tail drain).  Move excess waits onto
    preceding same-engine NoOps."""
    for f in nc.m.functions:
        for bb in f.blocks:
            insts = bb.instructions
            new_list = []
            changed = False
            for inst in insts:
                si = inst.sync_info
                waits = list(si.on_wait) if si else []
                if len(waits) > cap:
                    changed = True
                    for kk, w in enumerate(waits[:-cap]):
                        new_list.append(
                            mybir.InstNoOp(
                                name=f"{inst.name}-wsplit{kk}",
                                engine=inst.engine,
                                ins=[],
                                outs=[],
                                sync_info=mybir.SyncInfo(on_wait=[w], on_update=[]),
                            )
                        )
                    inst.sync_info = mybir.SyncInfo(
                        on_wait=waits[-cap:], on_update=list(si.on_update)
                    )
                new_list.append(inst)
            if changed:
                live = bb.instructions
                live.clear()
                for i in new_list:
                    bb.add_instruction(i)
    return nc


def build_gla(T=2048, groups=((0, 1, 2, 3), (4, 5, 6, 7)), n_devices=8,
              apply_beta=False, split_waits=True, use_silu=True):
    """Build the per-core SPMD program.  T = tokens per core."""
    assert T % 128 == 0
    NT = T // 128      # 128-token tiles
    KT = H // 128      # contraction slices

    nc = bass.Bass(num_devices=n_devices)
    xt_d = nc.declare_dram_parameter("xt", [H, T], BF16, isOutput=False)
    w_d = {
        n: nc.declare_dram_parameter(n, [H, H], BF16, isOutput=False)
        for n in ("wq", "wk", "wv", "wg", "wo")
    }
    beta_d = (
        nc.declare_dram_parameter("beta", [1, H], BF16, isOutput=False)
        if apply_beta
        else None
    )
    out_d = nc.declare_dram_parameter("out", [T, H], F32, isOutput=True)

    ks_in = nc.dram_tensor("ks_in", [1, H], F32)
    ks_out = nc.dram_tensor("ks_out", [1, H], F32)

    def mm(ps, lhsT, rhs, start, stop):
        nc.tensor.matmul(ps, lhsT=lhsT, rhs=rhs, start=start, stop=stop)

    with tile.TileContext(nc) as tc:
        with (
            tc.tile_pool(name="singles", bufs=1) as singles,
            tc.tile_pool(name="w", bufs=4) as wpool,
            tc.tile_pool(name="xt", bufs=1) as xtpool,
            tc.tile_pool(name="kt", bufs=1) as ktpool,
            tc.tile_pool(name="qt", bufs=1) as qtpool,
            tc.tile_pool(name="elu", bufs=2) as elupool,
            tc.tile_pool(name="prod", bufs=1) as prodpool,
            tc.tile_pool(name="small", bufs=3) as smpool,
            tc.tile_pool(name="z2", bufs=2) as zpool,
            tc.tile_pool(name="su", bufs=3) as supool,
            tc.tile_pool(name="ut", bufs=2) as utpool,
            tc.tile_pool(name="y", bufs=2) as ypool,
        ):
            ones_col = singles.tile([128, 1], BF16)
            nc.vector.memset(ones_col, 1.0)
            qk_all = singles.tile([128, NT, NH], F32)
            # rsqrt bit-hack constants (as APs: immediate ints on uint ops
            # are unreliable through the f32 immediate path)
            c_shift1 = singles.tile([128, 1], U32)
            nc.vector.memset(c_shift1, 1)
            c_magic = singles.tile([128, 1], U32)
            nc.vector.memset(c_magic, 0x5F3759DF)

            xt_all = xtpool.tile([128, KT, T], BF16)
            kt_all = ktpool.tile([128, NT, H], BF16)
            qt_all = qtpool.tile([128, NT, H], BF16)

            def alloc_w():
                return wpool.tile([128, KT, H], BF16, tag="w", name="wslot")

            def load_w(t, name, engine=None):
                # one wide descriptor: [H, H] viewed as [p, k-slice, cols]
                (engine or nc.sync).dma_start(
                    out=t[:, :, :],
                    in_=w_d[name][:, :].rearrange("(k p) n -> p k n", p=128))

            def elu1(dst, ps):
                # dst = elu(ps)+1 = min(exp(ps), 1) + relu(ps); ps is PSUM f32
                e = elupool.tile([128, 512], F32, tag="elue")
                r = elupool.tile([128, 512], F32, tag="elur")
                nc.scalar.activation(out=e, in_=ps, func=ACT_F.Exp)
                nc.scalar.activation(out=r, in_=ps, func=ACT_F.Relu)
                nc.vector.scalar_tensor_tensor(
                    out=dst, in0=e, scalar=1.0, in1=r,
                    op0=ALU.min, op1=ALU.add,
                )

            # stage the initial loads so tile 0's matmuls start after ~2 MB
            # instead of the full 8 MB X+Wk+Wq burst: X^T k-slices issue from
            # the Sync queue while Wk k-slices issue in parallel from the
            # (idle) scalar-engine queue; the bulk loads are single wide
            # descriptors (descriptor issue costs ~0.6us each).
            wk_t = alloc_w()
            for k in range(KT):
                ksl = slice(128 * k, 128 * (k + 1))
                nc.sync.dma_start(out=xt_all[:, k, 0:512],
                                  in_=xt_d[ksl, 0:512])
                nc.scalar.dma_start(out=wk_t[:, k, 0:512],
                                    in_=w_d["wk"][ksl, 0:512])
            nc.scalar.dma_start(
                out=wk_t[:, :, 512:1024],
                in_=w_d["wk"][:, 512:1024].rearrange("(k p) n -> p k n", p=128))
            nc.sync.dma_start(
                out=xt_all[:, :, 512:1024],
                in_=xt_d[:, 512:1024].rearrange("(k p) c -> p k c", p=128))
            nc.sync.dma_start(
                out=xt_all[:, :, 1024:T],
                in_=xt_d[:, 1024:T].rearrange("(k p) c -> p k c", p=128))
            wq_t = alloc_w()           # prefetched during phase 1a
            load_w(wq_t, "wq", engine=nc.scalar)

            # -------- phase 1a: k projection + k_sum (k kept in SBUF) ------
            with (
                tc.tile_pool(name="ks", bufs=1, space="PSUM") as kspool,
                tc.tile_pool(name="pk", bufs=2, space="PSUM") as pkpool,
            ):
                ks_ps = kspool.tile([1, H], F32)

                def emit_ksum(t):
                    for n in range(2):
                        nc.tensor.matmul(
                            ks_ps[:, 512 * n:512 * (n + 1)],
                            lhsT=ones_col,
                            rhs=kt_all[:, t, 512 * n:512 * (n + 1)],
                            start=(t == 0 and n == 0),
                            stop=(t == NT - 1 and n == 1),
                        )

                for t in range(NT):
                    for n in range(2):
                        pk = pkpool.tile([128, 512], F32, tag="pk")
                        nsl = slice(512 * n, 512 * (n + 1))
                        for k in range(KT):
                            lhs = xt_all[:, k, 128 * t:128 * (t + 1)]
                            mm(pk, lhs, wk_t[:, k, nsl], k == 0, k == KT - 1)
                        elu1(kt_all[:, t, nsl], pk)
                    # ksum of the previous tile: its elu chain finished while
                    # this tile's matmuls ran, so the PE never waits on DVE
                    if t > 0:
                        emit_ksum(t - 1)
                emit_ksum(NT - 1)
                # the AllReduce chain runs under high_priority: the Tile
                # scheduler otherwise parks the ks_sb copy ~30us deep into
                # phase 1b's vector work, which delays the collective enough
                # to stall the 1b->2 phase boundary by ~9us.
                with tc.high_priority():
                    ks_sb = singles.tile([1, H], F32)
                    nc.vector.tensor_copy(out=ks_sb, in_=ks_ps)
            with tc.high_priority():
                nc.sync.dma_start(out=ks_in[:, :], in_=ks_sb)
                nc.gpsimd.collective_compute(
                    "AllReduce", ALU.add,
                    replica_groups=[list(g) for g in groups],
                    ins=[ks_in[:, :]], outs=[ks_out[:, :]],
                )
                ksb_f32 = singles.tile([128, H], F32)
                nc.gpsimd.dma_start(out=ksb_f32,
                                    in_=ks_out[0:1, :].to_broadcast([128, H]))
                ksb = singles.tile([128, H], BF16)
                nc.gpsimd.tensor_copy(out=ksb, in_=ksb_f32)
            if apply_beta:
                beta_b = singles.tile([128, H], BF16)
                nc.gpsimd.dma_start(out=beta_b,
                                    in_=beta_d[0:1, :].to_broadcast([128, H]))

            # -------- phase 1b: q projection + qk (q kept in SBUF) ---------
            wv_t = alloc_w()           # prefetched for phase 2
            wg_t = alloc_w()
            load_w(wv_t, "wv")
            load_w(wg_t, "wg")
            with tc.tile_pool(name="pq", bufs=2, space="PSUM") as pqpool:
                for t in range(NT):
                    for n in range(2):
                        pq = pqpool.tile([128, 512], F32, tag="pq")
                        nsl = slice(512 * n, 512 * (n + 1))
                        for k in range(KT):
                            lhs = xt_all[:, k, 128 * t:128 * (t + 1)]
                            mm(pq, lhs, wq_t[:, k, nsl], k == 0, k == KT - 1)
                        elu1(qt_all[:, t, nsl], pq)
                    prod = prodpool.tile([128, H], BF16, tag="prod")
                    nc.vector.tensor_mul(prod, qt_all[:, t, :], kt_all[:, t, :])
                    nc.vector.reduce_sum(
                        out=qk_all[:, t, :],
                        in_=prod.rearrange("p (h d) -> p h d", d=DK),
                        axis=AX.X,
                    )

            # ---------------- phase 2: v, g, z, LN, gate, Wo ----------------
            wo_t = alloc_w()           # rotates into wk's slot (dead)
            load_w(wo_t, "wo")
            # pool creation order controls PSUM bank placement: py (first
            # needed ~25us into phase 2) takes the banks recycled from 1b's
            # pq pool, so the v/g matmuls (pa/pb) start on long-free banks
            # and don't wait for tile-15's elu chain to release pq.
            with (
                tc.tile_pool(name="py", bufs=2, space="PSUM") as pypool,
                tc.tile_pool(name="pa", bufs=3, space="PSUM") as papool,
                tc.tile_pool(name="pb", bufs=3, space="PSUM") as pbpool,
            ):
                def back_end(u, rsig, t):
                    # u^T via the DMA XBAR hardware transpose (2-byte dtypes
                    # only; verified layout ut[p,k,:] == u[:,128k+p]) -- keeps
                    # the 128 transposes/tile off the PE; 1/sigma is folded
                    # into the Wo PSUM->SBUF output move.
                    ut = utpool.tile([128, KT, 128], BF16, tag="ut")
                    nc.sync.dma_start_transpose(ut, u)
                    for n in range(2):
                        nsl = slice(512 * n, 512 * (n + 1))
                        py = pypool.tile([128, 512], F32, tag="py")
                        for k in range(KT):
                            mm(py, ut[:, k, :],
                               wo_t[:, k, nsl], k == 0, k == KT - 1)
                        y_sb = ypool.tile([128, 512], F32, tag="y")
                        if rsig is not None:
                            nc.vector.tensor_scalar(
                                out=y_sb, in0=py,
                                scalar1=rsig, scalar2=None, op0=ALU.mult,
                            )
                        else:
                            nc.vector.tensor_copy(out=y_sb, in_=py)
                        nc.sync.dma_start(
                            out=out_d[128 * t:128 * (t + 1), nsl], in_=y_sb)

                # 2-deep software pipeline: run tile t-2's transposes + Wo
                # while tiles t-1/t's DVE chains execute, so Vector-queue
                # scheduling jitter can never stall the PE.
                prevs = []
                for t in range(NT):
                    s_t = supool.tile([128, H], BF16, tag="s")
                    pvs = []
                    for n in range(2):
                        pv = papool.tile([128, 512], F32, tag="pa")
                        pg = pbpool.tile([128, 512], F32, tag="pb")
                        nsl = slice(512 * n, 512 * (n + 1))
                        for k in range(KT):
                            lhs = xt_all[:, k, 128 * t:128 * (t + 1)]
                            mm(pv, lhs, wv_t[:, k, nsl], k == 0, k == KT - 1)
                            mm(pg, lhs, wg_t[:, k, nsl], k == 0, k == KT - 1)
                        ssl = s_t[:, nsl]
                        if use_silu:
                            nc.scalar.activation(out=ssl, in_=pg, func=ACT_F.Silu)
                        else:  # CoreSim has no Silu table
                            nc.scalar.activation(out=ssl, in_=pg,
                                                 func=ACT_F.Sigmoid)
                            nc.vector.tensor_mul(ssl, ssl, pg)
                        pvs.append(pv)
                    # normalizer = per-head dot(q, k_sum)
                    nprod = prodpool.tile([128, H], BF16, tag="prod")
                    nc.vector.tensor_mul(nprod, qt_all[:, t, :], ksb)
                    norm = smpool.tile([128, NH], F32, tag="norm")
                    nc.vector.reduce_sum(
                        out=norm, in_=nprod.rearrange("p (h d) -> p h d", d=DK),
                        axis=AX.X,
                    )
                    rec = smpool.tile([128, NH], F32, tag="rec")
                    nc.vector.tensor_scalar_add(out=rec, in0=norm, scalar1=1e-6)
                    nc.vector.reciprocal(out=rec, in_=rec)
                    r = smpool.tile([128, NH], F32, tag="r")
                    nc.vector.tensor_mul(r, qk_all[:, t, :], rec)
                    # z = r (broadcast over d) * v
                    z = zpool.tile([128, H], BF16, tag="z")
                    for n in range(2):
                        rs = r[:, 8 * n:8 * (n + 1)]
                        r_b = bass.AP(tensor=rs.tensor, offset=rs.offset,
                                      ap=[list(rs.ap[0]), list(rs.ap[1]), [0, DK]])
                        nc.vector.tensor_tensor(
                            out=z[:, 512 * n:512 * (n + 1)],
                            in0=pvs[n], in1=r_b, op=ALU.mult,
                        )
                    # LayerNorm stats over the full 1024 features
                    st = smpool.tile([128, 2, nc.vector.BN_STATS_DIM], F32,
                                     tag="bnst")
                    for n in range(2):
                        nc.vector.bn_stats(out=st[:, n, :],
                                           in_=z[:, 512 * n:512 * (n + 1)])
                    mv = smpool.tile([128, nc.vector.BN_AGGR_DIM], F32, tag="mv")
                    nc.vector.bn_aggr(out=mv, in_=st)
                    # rsig = rsqrt(var + eps) on the DVE: exponent bit-hack
                    # seed + 2 Newton steps (max rel err ~5e-6).  Runs off the
                    # critical path; consumed only at Wo PSUM evacuation.
                    vq = smpool.tile([128, 1], F32, tag="vq")
                    nc.vector.tensor_scalar_add(out=vq, in0=mv[:, 1:2],
                                                scalar1=1e-5)
                    rsig = smpool.tile([128, 1], F32, tag="rsig")
                    nc.vector.tensor_scalar(
                        out=rsig.bitcast(U32), in0=vq.bitcast(U32),
                        scalar1=c_shift1[:, 0:1], scalar2=None,
                        op0=ALU.logical_shift_right,
                    )
                    nc.vector.tensor_tensor(
                        out=rsig.bitcast(U32), in0=c_magic,
                        in1=rsig.bitcast(U32), op=ALU.subtract,
                    )
                    nt1 = smpool.tile([128, 1], F32, tag="nt1")
                    for _ in range(2):
                        nc.vector.tensor_mul(nt1, rsig, rsig)
                        nc.vector.tensor_mul(nt1, nt1, vq)
                        nc.vector.tensor_scalar(
                            out=nt1, in0=nt1, scalar1=-0.5, scalar2=1.5,
                            op0=ALU.mult, op1=ALU.add,
                        )
                        nc.vector.tensor_mul(rsig, rsig, nt1)
                    # u = (z - mu) * silu(g); 1/sigma deferred past Wo
                    u = supool.tile([128, H], BF16, tag="u")
                    if apply_beta:
                        # beta breaks the deferral: apply rsig here instead
                        nc.vector.tensor_scalar(
                            out=u, in0=z, scalar1=mv[:, 0:1], scalar2=rsig,
                            op0=ALU.subtract, op1=ALU.mult,
                        )
                        nc.vector.tensor_add(out=u, in0=u, in1=beta_b)
                        nc.vector.tensor_mul(u, u, s_t)
                        rsig_eff = None
                    else:
                        nc.vector.tensor_scalar(
                            out=u, in0=z, scalar1=mv[:, 0:1], scalar2=None,
                            op0=ALU.subtract,
                        )
                        nc.vector.tensor_mul(u, u, s_t)
                        rsig_eff = rsig
                    prevs.append((u, rsig_eff, t))
                    if len(prevs) > 2:
                        back_end(*prevs.pop(0))
                for p in prevs:
                    back_end(*p)
    return _split_multi_waits(nc) if split_waits else nc


# ------------------------------------------------------------------
# host glue
# ------------------------------------------------------------------
_CACHE = {}
LAST_RESULT = None


def kernel(hidden_states, Wq, Wk, Wv, Wg, Wo, gamma, beta):
    import ml_dtypes
    bf16 = ml_dtypes.bfloat16

    hs = np.asarray(hidden_states, dtype=np.float32)
    Wq = np.asarray(Wq, dtype=np.float32)
    Wk = np.asarray(Wk, dtype=np.float32)
    Wv = np.asarray(Wv, dtype=np.float32)
    Wg = np.asarray(Wg, dtype=np.float32)
    Wo = np.asarray(Wo, dtype=np.float32)
    gamma = np.asarray(gamma, dtype=np.float32)
    beta = np.asarray(beta, dtype=np.float32)

    b, s, h = hs.shape
    tokens = hs.reshape(b * s, h)
    n_tok = b * s
    T = n_tok // N_CORES
    assert s % T == 0, "core token shards must not straddle batches"
    cores_per_batch = s // T

    groups = tuple(
        tuple(range(bi * cores_per_batch, (bi + 1) * cores_per_batch))
        for bi in range(b)
    )
    apply_beta = bool(np.any(beta))

    key = (T, groups, apply_beta)
    if key not in _CACHE:
        _CACHE[key] = build_gla(T=T, groups=groups, apply_beta=apply_beta)
    nc = _CACHE[key]

    wo_eff = (gamma[:, None] * Wo).astype(bf16)
    wq_b = Wq.astype(bf16)
    wk_b = Wk.astype(bf16)
    wv_b = Wv.astype(bf16)
    wg_b = Wg.astype(bf16)
    in_maps = []
    for i in range(N_CORES):
        m = {
            "xt": np.ascontiguousarray(tokens[i * T:(i + 1) * T].T).astype(bf16),
            "wq": wq_b, "wk": wk_b, "wv": wv_b, "wg": wg_b, "wo": wo_eff,
        }
        if apply_beta:
            m["beta"] = beta.reshape(1, h)
        in_maps.append(m)

    res = run_bass_kernel_spmd(
        nc, in_maps, core_ids=list(range(N_CORES)),
        trace=bool(os.environ.get("GLA_TRACE")),
    )
    global LAST_RESULT
    LAST_RESULT = res
    out = np.concatenate([res.results[i]["out"] for i in range(N_CORES)], axis=0)
    return out.reshape(b, s, h)
